# revision 4
# baseline (speedup 1.0000x reference)
"""Trainium2 Bass kernel for nn_AdjCompute (pairwise |x_i-x_j| -> 4x(1x1 conv+BN+lrelu) -> 1x1 conv).

Self-contained: hardcodes N=1536, C=64, H=[16,16,8,8], 8 NeuronCores,
row-sharding (192 rows/core). All activations stay SBUF-resident in fp16;
BN stats barriers use tiny AllGather collectives.

Layouts (per core):
  stage A (layers 1-2, 16ch): 24 groups of 8 rows. Activation buffers
    [128, 36864] fp16: partition = 16*r + o (r=row-in-group, o=channel),
    col = 1536*g + j.
  stage B (layers 3-4, 8ch): partition = 64*u + 8*r + o = 8*(8u+r) + o,
    col = 768*g + jj  (u = which 768-half of the group's 1536 cols).
"""

import numpy as np

from concourse import bacc, mybir, tile
from concourse.bass_utils import run_bass_kernel_spmd

NC_ = 8
N = 1536
C = 64
R = N // NC_  # 192 rows per core
G = R // 8  # 24 groups of 8 rows
W_A = G * N  # 36864 stage-A cols
W_B = G * (N // 2)  # 18432 stage-B cols
NTOT = float(N * N)
EPS = 1e-5
SLOPE = 0.01

f32, f16 = mybir.dt.float32, mybir.dt.float16
A = mybir.AluOpType
AF = mybir.ActivationFunctionType

_CACHE = {}
LAST_EXEC_NS = None


def _build():
    nc = bacc.Bacc("TRN2", target_bir_lowering=False, debug=False, num_devices=NC_)

    ext = {}

    def din(name, shape, dt):
        ext[name] = nc.dram_tensor(name, shape, dt, kind="ExternalInput")
        return ext[name]

    xT2_e = din("xT2", [128, N], f16)
    xp_e = din("xp", [128, 96], f32)
    xpn_e = din("xpn", [128, 96], f32)
    l1_e = din("lhsT1", [128, 32], f16)
    l2_e = din("lhsT2", [128, 128], f16)
    l3_e = din("lhsT3", [128, 64], f16)
    l4_e = din("lhsT4", [128, 128], f16)
    l5_e = din("lhsT5", [128, 16], f16)
    p16_e = din("pat16", [128, 128], f32)
    p8_e = din("pat8", [128, 128], f32)
    gb_e = din("gb", [128, 8], f32)  # cols: g1,be1,g2,be2,g3,be3,g4,be4 (broadcast)
    b5_e = din("b5b", [128, 1], f32)
    out_e = nc.dram_tensor("out", [R, N], f32, kind="ExternalOutput")

    with tile.TileContext(nc) as tc:
        with (
            tc.tile_pool(name="const", bufs=1) as cpool,
            tc.tile_pool(name="big", bufs=2) as big,
            tc.tile_pool(name="adjp", bufs=4) as adjp,
            tc.tile_pool(name="dtp", bufs=2) as dtp,
            tc.tile_pool(name="utp", bufs=2) as utp,
            tc.tile_pool(name="statp", bufs=1) as statp,
            tc.tile_pool(name="smallp", bufs=1) as smallp,
            tc.tile_pool(name="outp", bufs=1) as outp,
            tc.tile_pool(name="psA", bufs=4, space="PSUM") as psA,
            tc.tile_pool(name="psB", bufs=3, space="PSUM") as psB,
            tc.tile_pool(name="psS", bufs=1, space="PSUM") as psS,
            tc.tile_pool(name="dram", bufs=1, space="DRAM") as dram,
        ):
            # ---- load constants ----
            xT2 = cpool.tile([128, N], f16)
            xp = cpool.tile([128, 96], f32)
            xpn = cpool.tile([128, 96], f32)
            l1 = cpool.tile([128, 32], f16)
            l2 = cpool.tile([128, 128], f16)
            l3 = cpool.tile([128, 64], f16)
            l4 = cpool.tile([128, 128], f16)
            l5 = cpool.tile([128, 16], f16)
            p16 = cpool.tile([128, 128], f32)
            p8 = cpool.tile([128, 128], f32)
            gb = cpool.tile([128, 8], f32)
            b5b = cpool.tile([128, 1], f32)
            for t, e in [
                (xT2, xT2_e), (xp, xp_e), (xpn, xpn_e), (l1, l1_e), (l2, l2_e),
                (l3, l3_e), (l4, l4_e), (l5, l5_e), (p16, p16_e), (p8, p8_e),
                (gb, gb_e), (b5b, b5_e),
            ]:
                nc.sync.dma_start(t[(slice(None),) * len(t.shape)], e[(slice(None),) * len(t.shape)])

            # ---- activation storage ----
            h1 = big.tile([128, W_A], f16, tag="bigbuf")
            a1 = big.tile([128, W_A], f16, tag="bigbuf")

            st1 = statp.tile([128, 6 * 72], f32)
            st2 = statp.tile([128, 6 * 72], f32)
            st3 = statp.tile([128, 6 * 48], f32)
            st4 = statp.tile([128, 6 * 48], f32)

            # ================= PASS 1: adj -> h1, stats1 =================
            for g in range(G):
                adjs = []
                for pp in range(4):
                    idx = 4 * g + pp
                    adj = adjp.tile([128, N], f16, tag="adj", name=f"adj_{idx}")
                    if idx % 2 == 0:
                        nc.scalar.activation(
                            out=adj[:, :], in_=xT2[:, :], func=AF.Abs,
                            bias=xpn[:, idx : idx + 1], scale=1.0,
                        )
                    else:
                        d = dtp.tile([128, N], f16, tag="dt", name=f"d_{idx}")
                        nc.vector.tensor_scalar(
                            out=d[:, :], in0=xT2[:, :],
                            scalar1=xp[:, idx : idx + 1], scalar2=None,
                            op0=A.subtract,
                        )
                        nc.vector.scalar_tensor_tensor(
                            out=adj[:, :], in0=d[:, :], scalar=-1.0, in1=d[:, :],
                            op0=A.mult, op1=A.max,
                        )
                    adjs.append(adj)
                for s in range(3):
                    ps = psA.tile([128, 512], f32, tag="psA", name=f"h1p_{g}_{s}")
                    for pp in range(4):
                        nc.tensor.matmul(
                            ps[32 * pp : 32 * pp + 32, :],
                            l1[:, :],
                            adjs[pp][:, 512 * s : 512 * s + 512],
                            start=True, stop=True,
                            tile_position=(0, 32 * pp),
                        )
                    t = 3 * g + s
                    nc.vector.bn_stats(st1[:, 6 * t : 6 * t + 6], ps[:, :])
                    nc.scalar.activation(
                        out=h1[:, 1536 * g + 512 * s : 1536 * g + 512 * s + 512],
                        in_=ps[:, :], func=AF.Copy,
                    )

            # ---- barrier helper ----
            def barrier(k, stbuf, ntiles, pat, np_count, gcol, becol):
                ba = smallp.tile([128, 2], f32, name=f"ba{k}")
                nc.vector.bn_aggr(ba[:, :], stbuf[:, : 6 * ntiles])
                m2 = smallp.tile([128, 1], f32, name=f"m2_{k}")
                nc.scalar.activation(out=m2[:, :], in_=ba[:, 0:1], func=AF.Square)
                sq = smallp.tile([128, 2], f32, name=f"sq{k}")
                nc.vector.tensor_scalar(
                    out=sq[:, 0:1], in0=ba[:, 0:1], scalar1=float(np_count),
                    scalar2=None, op0=A.mult,
                )
                q = smallp.tile([128, 1], f32, name=f"q{k}")
                nc.vector.tensor_tensor(
                    out=q[:, :], in0=ba[:, 1:2], in1=m2[:, :], op=A.add,
                )
                nc.vector.tensor_scalar(
                    out=sq[:, 1:2], in0=q[:, :], scalar1=float(np_count),
                    scalar2=None, op0=A.mult,
                )
                pf = psS.tile([128, 2], f32, tag="psS", name=f"pf{k}")
                nc.tensor.matmul(pf[:, :], pat[:, :], sq[:, :], start=True, stop=True)
                gl = smallp.tile([128, 2], f32, name=f"gl{k}")
                nc.scalar.activation(out=gl[:, :], in_=pf[:, :], func=AF.Copy)
                agi = dram.tile([128, 2], f32, name=f"agi{k}")
                ago = dram.tile([128 * NC_, 2], f32, addr_space="Shared", name=f"ago{k}")
                nc.sync.dma_start(agi[:, :], gl[:, :])
                nc.gpsimd.collective_compute(
                    "AllGather", A.bypass,
                    replica_groups=[list(range(NC_))],
                    ins=[agi.opt()], outs=[ago.opt()],
                )
                agv = smallp.tile([128, 2, NC_], f32, name=f"agv{k}")
                nc.sync.dma_start(
                    agv[:, :, :], ago.rearrange("(b p) c -> p c b", b=NC_),
                )
                gt = smallp.tile([128, 2], f32, name=f"gt{k}")
                nc.vector.tensor_reduce(
                    out=gt[:, :], in_=agv[:, :, :],
                    axis=mybir.AxisListType.X, op=A.add,
                )
                negmean = smallp.tile([128, 1], f32, name=f"nm{k}")
                nc.vector.tensor_scalar(
                    out=negmean[:, :], in0=gt[:, 0:1], scalar1=-1.0 / NTOT,
                    scalar2=None, op0=A.mult,
                )
                ex2e = smallp.tile([128, 1], f32, name=f"ex{k}")
                nc.vector.tensor_scalar(
                    out=ex2e[:, :], in0=gt[:, 1:2], scalar1=1.0 / NTOT,
                    scalar2=EPS, op0=A.mult, op1=A.add,
                )
                msq = smallp.tile([128, 1], f32, name=f"ms{k}")
                nc.scalar.activation(out=msq[:, :], in_=negmean[:, :], func=AF.Square)
                vpe = smallp.tile([128, 1], f32, name=f"vp{k}")
                nc.vector.scalar_tensor_tensor(
                    out=vpe[:, :], in0=msq[:, :], scalar=-1.0, in1=ex2e[:, :],
                    op0=A.mult, op1=A.add,
                )
                rinv = smallp.tile([128, 1], f32, name=f"ri{k}")
                nc.vector.reciprocal(rinv[:, :], vpe[:, :])
                rstd = smallp.tile([128, 1], f32, name=f"rs{k}")
                nc.scalar.activation(out=rstd[:, :], in_=rinv[:, :], func=AF.Sqrt)
                sk = smallp.tile([128, 1], f32, name=f"s{k}")
                nc.vector.tensor_tensor(
                    out=sk[:, :], in0=rstd[:, :], in1=gb[:, gcol : gcol + 1], op=A.mult,
                )
                tk = smallp.tile([128, 1], f32, name=f"t{k}")
                nc.vector.scalar_tensor_tensor(
                    out=tk[:, :], in0=sk[:, :], scalar=negmean[:, :],
                    in1=gb[:, becol : becol + 1], op0=A.mult, op1=A.add,
                )
                return sk, tk

            s1, t1 = barrier(1, st1, 72, p16, 72 * 512, 0, 1)

            # ================= PASS 2: apply1 -> a1, mm2, stats2 =================
            for g in range(G):
                nc.scalar.activation(
                    out=a1[:, 1536 * g : 1536 * g + 1536],
                    in_=h1[:, 1536 * g : 1536 * g + 1536],
                    func=AF.Lrelu, scale=s1[:, :], bias=t1[:, :], alpha=SLOPE,
                )
                for s in range(3):
                    ps = psA.tile([128, 512], f32, tag="psA", name=f"h2p_{g}_{s}")
                    nc.tensor.matmul(
                        ps[:, :], l2[:, :],
                        a1[:, 1536 * g + 512 * s : 1536 * g + 512 * s + 512],
                        start=True, stop=True,
                    )
                    t = 3 * g + s
                    nc.vector.bn_stats(st2[:, 6 * t : 6 * t + 6], ps[:, :])

            s2, t2 = barrier(2, st2, 72, p16, 72 * 512, 2, 3)

            # ================= PASS 3: remat h2, apply2 -> a2, mm3, stats3 ======
            a2 = big.tile([128, W_A], f16, tag="bigbuf")
            for g in range(G):
                for s in range(3):
                    ps = psA.tile([128, 512], f32, tag="psA", name=f"h2r_{g}_{s}")
                    nc.tensor.matmul(
                        ps[:, :], l2[:, :],
                        a1[:, 1536 * g + 512 * s : 1536 * g + 512 * s + 512],
                        start=True, stop=True,
                    )
                    t = 3 * g + s
                    dst = a2[:, 1536 * g + 512 * s : 1536 * g + 512 * s + 512]
                    if t % 8 == 7:
                        u = utp.tile([128, 512], f16, tag="ut", name=f"u2_{t}")
                        nc.vector.tensor_scalar(
                            out=u[:, :], in0=ps[:, :], scalar1=s2[:, :],
                            scalar2=t2[:, :], op0=A.mult, op1=A.add,
                        )
                        nc.vector.scalar_tensor_tensor(
                            out=dst, in0=u[:, :], scalar=SLOPE, in1=u[:, :],
                            op0=A.mult, op1=A.max,
                        )
                    else:
                        nc.scalar.activation(
                            out=dst, in_=ps[:, :], func=AF.Lrelu,
                            scale=s2[:, :], bias=t2[:, :], alpha=SLOPE,
                        )
                for w in range(2):
                    ps3 = psB.tile([128, 384], f32, tag="psB", name=f"h3p_{g}_{w}")
                    for u in range(2):
                        nc.tensor.matmul(
                            ps3[64 * u : 64 * u + 64, :],
                            l3[:, :],
                            a2[:, 1536 * g + 768 * u + 384 * w : 1536 * g + 768 * u + 384 * w + 384],
                            start=True, stop=True,
                            tile_position=(0, 64 * u),
                        )
                    t3 = 2 * g + w
                    nc.vector.bn_stats(st3[:, 6 * t3 : 6 * t3 + 6], ps3[:, :])

            s3, t3v = barrier(3, st3, 48, p8, 48 * 384, 4, 5)

            # ================= PASS 4: remat h3, apply3 -> a3, mm4, stats4 ======
            a3 = big.tile([128, W_B], f16, tag="bigbuf")
            for g in range(G):
                for w in range(2):
                    ps3 = psB.tile([128, 384], f32, tag="psB", name=f"h3r_{g}_{w}")
                    for u in range(2):
                        nc.tensor.matmul(
                            ps3[64 * u : 64 * u + 64, :],
                            l3[:, :],
                            a2[:, 1536 * g + 768 * u + 384 * w : 1536 * g + 768 * u + 384 * w + 384],
                            start=True, stop=True,
                            tile_position=(0, 64 * u),
                        )
                    dst = a3[:, 768 * g + 384 * w : 768 * g + 384 * w + 384]
                    nc.scalar.activation(
                        out=dst, in_=ps3[:, :], func=AF.Lrelu,
                        scale=s3[:, :], bias=t3v[:, :], alpha=SLOPE,
                    )
                    ps4 = psB.tile([128, 384], f32, tag="psB", name=f"h4p_{g}_{w}")
                    nc.tensor.matmul(
                        ps4[:, :], l4[:, :], dst, start=True, stop=True,
                    )
                    t4 = 2 * g + w
                    nc.vector.bn_stats(st4[:, 6 * t4 : 6 * t4 + 6], ps4[:, :])

            s4, t4v = barrier(4, st4, 48, p8, 48 * 384, 6, 7)

            # ================= PASS 5: remat h4, apply4 -> a4, mm5, out ========
            a4 = big.tile([128, W_B], f16, tag="bigbuf")
            outb = outp.tile([128, 12, 384], f32)
            for m in range(12):
                ps5 = psB.tile([128, 384], f32, tag="psB", name=f"h5p_{m}")
                for k in range(4):
                    g = 2 * m + k // 2
                    w = k % 2
                    ps4 = psB.tile([128, 384], f32, tag="psB", name=f"h4r_{m}_{k}")
                    nc.tensor.matmul(
                        ps4[:, :], l4[:, :],
                        a3[:, 768 * g + 384 * w : 768 * g + 384 * w + 384],
                        start=True, stop=True,
                    )
                    dst = a4[:, 768 * g + 384 * w : 768 * g + 384 * w + 384]
                    t = 2 * g + w
                    if t % 3 == 2:
                        u = utp.tile([128, 384], f16, tag="ut", name=f"u4_{t}")
                        nc.vector.tensor_scalar(
                            out=u[:, :], in0=ps4[:, :], scalar1=s4[:, :],
                            scalar2=t4v[:, :], op0=A.mult, op1=A.add,
                        )
                        nc.vector.scalar_tensor_tensor(
                            out=dst, in0=u[:, :], scalar=SLOPE, in1=u[:, :],
                            op0=A.mult, op1=A.max,
                        )
                    else:
                        nc.scalar.activation(
                            out=dst, in_=ps4[:, :], func=AF.Lrelu,
                            scale=s4[:, :], bias=t4v[:, :], alpha=SLOPE,
                        )
                    nc.tensor.matmul(
                        ps5[32 * k : 32 * k + 16, :], l5[:, :], dst,
                        start=True, stop=True,
                        tile_position=(0, 32 * k),
                    )
                nc.scalar.activation(
                    out=outb[:, m, :], in_=ps5[:, :],
                    func=AF.Identity, bias=b5b[:, :], scale=1.0,
                )

            # out[192,1536]: rows decompose as (m:12, gg:2, r:8) with g = 2m+gg,
            # cols as (u:2, w:2, jj:384). outb partition = 32*(2gg+w) + 8u + r.
            out_view = out_e.rearrange(
                "(m gg r) (u w jj) -> gg w u r m jj", m=12, gg=2, r=8, u=2, w=2, jj=384
            )
            for gg in range(2):
                for w in range(2):
                    for u in range(2):
                        pb = 32 * (2 * gg + w) + 8 * u
                        nc.sync.dma_start(
                            out_view[gg, w, u], outb[pb : pb + 8, :, :]
                        )

    nc.compile()
    return nc


def _host_inputs(x, W1, W2, W3, W4, W5, g1, be1, g2, be2, g3, be3, g4, be4, b5):
    xT2 = np.concatenate([x.T, x.T], axis=0).astype(np.float16)  # [128, N]

    lhsT1 = np.zeros((128, 32), np.float32)
    for d in range(2):
        lhsT1[64 * d : 64 * d + 64, 16 * d : 16 * d + 16] = W1.T
    lhsT2 = np.zeros((128, 128), np.float32)
    for r in range(8):
        lhsT2[16 * r : 16 * r + 16, 16 * r : 16 * r + 16] = W2.T
    lhsT3 = np.zeros((128, 64), np.float32)
    for r in range(8):
        lhsT3[16 * r : 16 * r + 16, 8 * r : 8 * r + 8] = W3.T
    lhsT4 = np.zeros((128, 128), np.float32)
    for b in range(16):
        lhsT4[8 * b : 8 * b + 8, 8 * b : 8 * b + 8] = W4.T
    lhsT5 = np.zeros((128, 16), np.float32)
    for b in range(16):
        lhsT5[8 * b : 8 * b + 8, b] = W5[0, :]

    q = np.arange(128)
    pat16 = (q[:, None] % 16 == q[None, :] % 16).astype(np.float32)
    pat8 = (q[:, None] % 8 == q[None, :] % 8).astype(np.float32)
    gb = np.stack(
        [
            g1[q % 16], be1[q % 16], g2[q % 16], be2[q % 16],
            g3[q % 8], be3[q % 8], g4[q % 8], be4[q % 8],
        ],
        axis=1,
    ).astype(np.float32)
    b5b = np.full((128, 1), float(b5[0]), np.float32)

    common = {
        "xT2": xT2,
        "lhsT1": lhsT1.astype(np.float16),
        "lhsT2": lhsT2.astype(np.float16),
        "lhsT3": lhsT3.astype(np.float16),
        "lhsT4": lhsT4.astype(np.float16),
        "lhsT5": lhsT5.astype(np.float16),
        "pat16": pat16,
        "pat8": pat8,
        "gb": gb,
        "b5b": b5b,
    }

    in_maps = []
    for core in range(NC_):
        rows = x[R * core : R * core + R]  # [192, 64]
        xp = np.empty((128, 96), np.float32)
        xp[0:64, :] = rows[0::2].T
        xp[64:128, :] = rows[1::2].T
        m = dict(common)
        m["xp"] = xp
        m["xpn"] = -xp
        in_maps.append(m)
    return in_maps


def kernel(**inputs):
    global LAST_EXEC_NS
    import os

    x = np.asarray(inputs["x"], np.float32)
    args = [
        np.asarray(inputs[k], np.float32)
        for k in ("W1", "W2", "W3", "W4", "W5", "g1", "be1", "g2", "be2",
                  "g3", "be3", "g4", "be4", "b5")
    ]
    in_maps = _host_inputs(x, *args)

    if "nc" not in _CACHE:
        _CACHE["nc"] = _build()
    nc = _CACHE["nc"]

    trace = os.environ.get("KERNEL_TRACE", "0") == "1"
    res = run_bass_kernel_spmd(
        nc, in_maps, core_ids=list(range(NC_)), trace=trace
    )
    LAST_EXEC_NS = res.exec_time_ns
    out = np.concatenate([res.results[c]["out"] for c in range(NC_)], axis=0)
    return out.astype(np.float32)


# revision 8
# speedup vs baseline: 1.3527x; 1.3527x over previous
"""Trainium2 Bass kernel for nn_AdjCompute (pairwise |x_i-x_j| -> 4x(1x1 conv+BN+lrelu) -> 1x1 conv).

v2: wrapped-band symmetric version. out[i,j] == out[j,i], so each 8-row group
g computes only a cyclic column window [8g, 8g + W_g) mod 1536 with
W_g = 776 for g < 96 and 768 for g >= 96. This covers every off-diagonal
8x8 block pair exactly once and every diagonal block fully; the mirror half
is assembled on the host. BN statistics use
  S_full = 2*S_computed - S_diagblocks.
All cores get identical op shapes (12 groups of each width class); per-core
variation (which rows, which wrapped columns) is carried entirely by input
data (xw = pre-gathered wrapped x columns, xp = pair scalars).

Device layout (per core, 24 groups, total computed cols WTA = 18528):
  stage A (64->16->16 ch): flat column stream; group gi at stream cols
    [OFF[gi], OFF[gi]+W), partition = 16*r + o (row-in-group, channel).
  stage B (16->8->8->1 ch): stream halves stacked: partition = 64*u + 8*r + o,
    stage-B col c in [0, 9264): u=0 <-> stage-A col c, u=1 <-> 9264 + c.
Output: raw [128, 2688] f32 stage-B stream dump per core; host unscrambles
and mirrors.
"""

import numpy as np

from concourse import bacc, mybir, tile
from concourse.bass_utils import run_bass_kernel_spmd

NC_ = 8
N = 1536
NTOT = float(N * N)
EPS = 1e-5
SLOPE = 0.01
GPC = 24  # groups per core

f32, f16 = mybir.dt.float32, mybir.dt.float16
A = mybir.AluOpType
AF = mybir.ActivationFunctionType

_CACHE = {}
LAST_EXEC_NS = None


def _glist(core):
    gl = []
    for t in range(12):
        gl.append(core + 8 * t)  # W = 776
        gl.append(96 + core + 8 * t)  # W = 768
    return gl


_LL = [776 if i % 2 == 0 else 768 for i in range(GPC)]  # identical for all cores
_OFF = np.concatenate([[0], np.cumsum(_LL)]).astype(int)
WTA = int(_OFF[-1])  # 18528
WTB = WTA // 2  # 9264
assert int(_OFF[12]) == WTB

# stage-A per-group tiling (chunks of <=512, one PSUM bank) for mm1/copy/stats
TILE_A = []  # (gi, stream_start, width)
for gi in range(GPC):
    L = _LL[gi]
    c = 0
    while c < L:
        w = min(512, L - c)
        TILE_A.append((gi, int(_OFF[gi]) + c, w))
        c += w
NTA = len(TILE_A)  # 48

# flat stage-A tiling for mm2 (512 chunks)
TILE_F = []
c = 0
while c < WTA:
    w = min(512, WTA - c)
    TILE_F.append((c, w))
    c += w
NTF = len(TILE_F)  # 37

SLAB_A = []
c = 0
while c < WTA:
    w = min(1536, WTA - c)
    SLAB_A.append((c, w))
    c += w

# stage-B tiling (384 chunks)
TILE_B = []
c = 0
while c < WTB:
    w = min(384, WTB - c)
    TILE_B.append((c, w))
    c += w
NTB = len(TILE_B)  # 25
NP5 = (NTB + 3) // 4  # 7 psum5 tiles
WOUT = NP5 * 384  # 2688

SLAB_B = []
c = 0
while c < WTB:
    w = min(1536, WTB - c)
    SLAB_B.append((c, w))
    c += w


def _build():
    nc = bacc.Bacc("TRN2", target_bir_lowering=False, debug=False, num_devices=NC_)

    def din(name, shape, dt):
        return nc.dram_tensor(name, shape, dt, kind="ExternalInput")

    xw_e = din("xw", [128, WTA], f16)
    xp_e = din("xp", [128, 96], f32)
    xpn_e = din("xpn", [128, 96], f32)
    l1_e = din("lhsT1", [128, 32], f16)
    l2_e = din("lhsT2", [128, 128], f16)
    l3_e = din("lhsT3", [128, 64], f16)
    l4_e = din("lhsT4", [128, 128], f16)
    l5_e = din("lhsT5", [128, 16], f16)
    p16_e = din("pat16", [128, 128], f32)
    p8_e = din("pat8", [128, 128], f32)
    gb_e = din("gb", [128, 8], f32)
    b5_e = din("b5b", [128, 1], f32)
    out_e = nc.dram_tensor("out", [128, WOUT], f32, kind="ExternalOutput")

    with tile.TileContext(nc) as tc:
        with (
            tc.tile_pool(name="const", bufs=1) as cpool,
            tc.tile_pool(name="big", bufs=3) as big,
            tc.tile_pool(name="adjp", bufs=4) as adjp,
            tc.tile_pool(name="dtp", bufs=2) as dtp,
            tc.tile_pool(name="atp", bufs=3) as atp,
            tc.tile_pool(name="jkp", bufs=2) as jkp,
            tc.tile_pool(name="statp", bufs=1) as statp,
            tc.tile_pool(name="smallp", bufs=1) as smallp,
            tc.tile_pool(name="outp", bufs=1) as outp,
            tc.tile_pool(name="psA", bufs=4, space="PSUM") as psA,
            tc.tile_pool(name="psB", bufs=3, space="PSUM") as psB,
            tc.tile_pool(name="psS", bufs=1, space="PSUM") as psS,
            tc.tile_pool(name="dram", bufs=1, space="DRAM") as dram,
        ):
            # ---- constants ----
            xp = cpool.tile([128, 96], f32)
            xpn = cpool.tile([128, 96], f32)
            l1 = cpool.tile([128, 32], f16)
            l2 = cpool.tile([128, 128], f16)
            l3 = cpool.tile([128, 64], f16)
            l4 = cpool.tile([128, 128], f16)
            l5 = cpool.tile([128, 16], f16)
            p16 = cpool.tile([128, 128], f32)
            p8 = cpool.tile([128, 128], f32)
            gb = cpool.tile([128, 8], f32)
            b5b = cpool.tile([128, 1], f32)
            for t, e in [
                (xp, xp_e), (xpn, xpn_e), (l1, l1_e), (l2, l2_e),
                (l3, l3_e), (l4, l4_e), (l5, l5_e), (p16, p16_e), (p8, p8_e),
                (gb, gb_e), (b5b, b5_e),
            ]:
                sl = (slice(None),) * len(t.shape)
                nc.sync.dma_start(t[sl], e[sl])

            # wrapped x-column stream (big pool slot, freed after pass 1)
            xw = big.tile([128, WTA], f16, tag="hbuf")
            for gi in range(GPC):
                o0, o1 = int(_OFF[gi]), int(_OFF[gi + 1])
                nc.sync.dma_start(xw[:, o0:o1], xw_e[:, o0:o1])

            h1 = big.tile([128, WTA], f16, tag="hbuf")

            sumb = {}
            sqb = {}
            dsb = {}
            dqb = {}
            for k, nt in [(1, NTA), (2, NTF), (3, NTB), (4, NTB)]:
                sumb[k] = statp.tile([128, nt], f32, name=f"sumb{k}")
                sqb[k] = statp.tile([128, nt], f32, name=f"sqb{k}")
                dsb[k] = statp.tile([128, GPC], f32, name=f"dsb{k}")
                dqb[k] = statp.tile([128, GPC], f32, name=f"dqb{k}")
                nc.vector.memset(dsb[k][:, :], 0.0)
                nc.vector.memset(dqb[k][:, :], 0.0)

            def copy_and_stats(k, ti, ps, wid, dst, eng):
                # copy psum->sbuf f16 with fused sum accum + a sumsq pass
                if eng == 0:
                    nc.vector.tensor_scalar(
                        out=dst, in0=ps, scalar1=1.0, scalar2=0.0,
                        op0=A.mult, op1=A.add,
                        accum_out=sumb[k][:, ti : ti + 1],
                    )
                    jk = jkp.tile([128, 512], f16, tag="jk", name=f"jk{k}_{ti}")
                    nc.scalar.activation(
                        out=jk[:, :wid], in_=ps, func=AF.Square,
                        accum_out=sqb[k][:, ti : ti + 1],
                    )
                else:
                    nc.scalar.activation(
                        out=dst, in_=ps, func=AF.Identity,
                        accum_out=sumb[k][:, ti : ti + 1],
                    )
                    jk = jkp.tile([128, 512], f16, tag="jk", name=f"jk{k}_{ti}")
                    nc.vector.scalar_tensor_tensor(
                        out=jk[:, :wid], in0=dst, scalar=0.0, in1=dst,
                        op0=A.add, op1=A.mult,
                        accum_out=sqb[k][:, ti : ti + 1],
                    )

            def diag_stats(k, gi, hst, c0, p0, pn):
                jd = smallp.tile([128, 8], f16, name=f"jd{k}_{gi}", tag="jd")
                nc.vector.tensor_scalar(
                    out=jd[p0 : p0 + pn, :], in0=hst[p0 : p0 + pn, c0 : c0 + 8],
                    scalar1=1.0, scalar2=0.0, op0=A.mult, op1=A.add,
                    accum_out=dsb[k][p0 : p0 + pn, gi : gi + 1],
                )
                jd2 = smallp.tile([128, 8], f16, name=f"jd2{k}_{gi}", tag="jd2")
                nc.scalar.activation(
                    out=jd2[p0 : p0 + pn, :], in_=hst[p0 : p0 + pn, c0 : c0 + 8],
                    func=AF.Square,
                    accum_out=dqb[k][p0 : p0 + pn, gi : gi + 1],
                )

            # ================= PASS 1 =================
            gi2tiles = {}
            for ti, (gi, c0, w) in enumerate(TILE_A):
                gi2tiles.setdefault(gi, []).append((ti, c0, w))

            for gi in range(GPC):
                L = _LL[gi]
                o0 = int(_OFF[gi])
                adjs = []
                for pp in range(4):
                    idx = 4 * gi + pp
                    adj = adjp.tile([128, 776], f16, tag="adj", name=f"adj_{idx}")
                    if idx % 2 == 0:
                        nc.scalar.activation(
                            out=adj[:, :L], in_=xw[:, o0 : o0 + L], func=AF.Abs,
                            bias=xpn[:, idx : idx + 1], scale=1.0,
                        )
                    else:
                        d = dtp.tile([128, 776], f16, tag="dt", name=f"d_{idx}")
                        nc.vector.tensor_scalar(
                            out=d[:, :L], in0=xw[:, o0 : o0 + L],
                            scalar1=xp[:, idx : idx + 1], scalar2=None,
                            op0=A.subtract,
                        )
                        nc.vector.scalar_tensor_tensor(
                            out=adj[:, :L], in0=d[:, :L], scalar=-1.0, in1=d[:, :L],
                            op0=A.mult, op1=A.max,
                        )
                    adjs.append(adj)
                for ti, c0, w in gi2tiles[gi]:
                    lc = c0 - o0
                    ps = psA.tile([128, 512], f32, tag="psA", name=f"h1p_{ti}")
                    for pp in range(4):
                        nc.tensor.matmul(
                            ps[32 * pp : 32 * pp + 32, :w],
                            l1[:, :],
                            adjs[pp][:, lc : lc + w],
                            start=True, stop=True,
                            tile_position=(0, 32 * pp),
                        )
                    copy_and_stats(1, ti, ps[:, :w], w, h1[:, c0 : c0 + w], ti % 2)
                diag_stats(1, gi, h1, o0, 0, 128)

            # ---- barrier ----
            def barrier(k, ntiles, pat, gcol, becol):
                s_l = smallp.tile([128, 1], f32, name=f"sl{k}")
                nc.vector.tensor_reduce(
                    out=s_l[:, :], in_=sumb[k][:, :ntiles],
                    axis=mybir.AxisListType.X, op=A.add,
                )
                q_l = smallp.tile([128, 1], f32, name=f"ql{k}")
                nc.vector.tensor_reduce(
                    out=q_l[:, :], in_=sqb[k][:, :ntiles],
                    axis=mybir.AxisListType.X, op=A.add,
                )
                ds_l = smallp.tile([128, 1], f32, name=f"dsl{k}")
                nc.vector.tensor_reduce(
                    out=ds_l[:, :], in_=dsb[k][:, :],
                    axis=mybir.AxisListType.X, op=A.add,
                )
                dq_l = smallp.tile([128, 1], f32, name=f"dql{k}")
                nc.vector.tensor_reduce(
                    out=dq_l[:, :], in_=dqb[k][:, :],
                    axis=mybir.AxisListType.X, op=A.add,
                )
                # (2*S - DS)/2 = S - DS/2 ; the 2/NTOT scale is applied below
                sq = smallp.tile([128, 2], f32, name=f"sq{k}")
                nc.vector.scalar_tensor_tensor(
                    out=sq[:, 0:1], in0=ds_l[:, :], scalar=-0.5, in1=s_l[:, :],
                    op0=A.mult, op1=A.add,
                )
                nc.vector.scalar_tensor_tensor(
                    out=sq[:, 1:2], in0=dq_l[:, :], scalar=-0.5, in1=q_l[:, :],
                    op0=A.mult, op1=A.add,
                )
                pf = psS.tile([128, 2], f32, tag="psS", name=f"pf{k}")
                nc.tensor.matmul(pf[:, :], pat[:, :], sq[:, :], start=True, stop=True)
                gl = smallp.tile([128, 2], f32, name=f"gl{k}")
                nc.scalar.activation(out=gl[:, :], in_=pf[:, :], func=AF.Copy)
                agi = dram.tile([128, 2], f32, name=f"agi{k}")
                ago = dram.tile([128 * NC_, 2], f32, addr_space="Shared", name=f"ago{k}")
                nc.sync.dma_start(agi[:, :], gl[:, :])
                nc.gpsimd.collective_compute(
                    "AllGather", A.bypass,
                    replica_groups=[list(range(NC_))],
                    ins=[agi.opt()], outs=[ago.opt()],
                )
                agv = smallp.tile([128, 2, NC_], f32, name=f"agv{k}")
                nc.sync.dma_start(
                    agv[:, :, :], ago.rearrange("(b p) c -> p c b", b=NC_),
                )
                gt = smallp.tile([128, 2], f32, name=f"gt{k}")
                nc.vector.tensor_reduce(
                    out=gt[:, :], in_=agv[:, :, :],
                    axis=mybir.AxisListType.X, op=A.add,
                )
                negmean = smallp.tile([128, 1], f32, name=f"nm{k}")
                nc.vector.tensor_scalar(
                    out=negmean[:, :], in0=gt[:, 0:1], scalar1=-2.0 / NTOT,
                    scalar2=None, op0=A.mult,
                )
                ex2e = smallp.tile([128, 1], f32, name=f"ex{k}")
                nc.vector.tensor_scalar(
                    out=ex2e[:, :], in0=gt[:, 1:2], scalar1=2.0 / NTOT,
                    scalar2=EPS, op0=A.mult, op1=A.add,
                )
                msq = smallp.tile([128, 1], f32, name=f"ms{k}")
                nc.scalar.activation(out=msq[:, :], in_=negmean[:, :], func=AF.Square)
                vpe = smallp.tile([128, 1], f32, name=f"vp{k}")
                nc.vector.scalar_tensor_tensor(
                    out=vpe[:, :], in0=msq[:, :], scalar=-1.0, in1=ex2e[:, :],
                    op0=A.mult, op1=A.add,
                )
                rinv = smallp.tile([128, 1], f32, name=f"ri{k}")
                nc.vector.reciprocal(rinv[:, :], vpe[:, :])
                rstd = smallp.tile([128, 1], f32, name=f"rs{k}")
                nc.scalar.activation(out=rstd[:, :], in_=rinv[:, :], func=AF.Sqrt)
                sk = smallp.tile([128, 1], f32, name=f"s{k}")
                nc.vector.tensor_tensor(
                    out=sk[:, :], in0=rstd[:, :], in1=gb[:, gcol : gcol + 1], op=A.mult,
                )
                tk = smallp.tile([128, 1], f32, name=f"t{k}")
                nc.vector.scalar_tensor_tensor(
                    out=tk[:, :], in0=sk[:, :], scalar=negmean[:, :],
                    in1=gb[:, becol : becol + 1], op0=A.mult, op1=A.add,
                )
                return sk, tk

            s1, t1 = barrier(1, NTA, p16, 0, 1)

            # ================= PASS 2: apply1, mm2, h2 =================
            h2 = big.tile([128, WTA], f16, tag="hbuf")
            for si, (c0, w) in enumerate(SLAB_A):
                at = atp.tile([128, 1536], f16, tag="at", name=f"a1_{si}")
                nc.scalar.activation(
                    out=at[:, :w], in_=h1[:, c0 : c0 + w],
                    func=AF.Lrelu, scale=s1[:, :], bias=t1[:, :], alpha=SLOPE,
                )
                for z in range(0, w, 512):
                    wz = min(512, w - z)
                    ti = (c0 + z) // 512
                    ps = psA.tile([128, 512], f32, tag="psA", name=f"h2p_{ti}")
                    nc.tensor.matmul(
                        ps[:, :wz], l2[:, :], at[:, z : z + wz],
                        start=True, stop=True,
                    )
                    copy_and_stats(
                        2, ti, ps[:, :wz], wz, h2[:, c0 + z : c0 + z + wz], ti % 2
                    )
            for gi in range(GPC):
                diag_stats(2, gi, h2, int(_OFF[gi]), 0, 128)

            s2, t2 = barrier(2, NTF, p16, 2, 3)

            # ================= PASS 3: apply2, mm3, h3 =================
            a2 = big.tile([128, WTA], f16, tag="hbuf")
            for si, (c0, w) in enumerate(SLAB_A):
                if si % 2 == 0:
                    nc.scalar.activation(
                        out=a2[:, c0 : c0 + w], in_=h2[:, c0 : c0 + w],
                        func=AF.Lrelu, scale=s2[:, :], bias=t2[:, :], alpha=SLOPE,
                    )
                else:
                    u = dtp.tile([128, 1536], f16, tag="dt2", name=f"u2_{si}")
                    nc.vector.tensor_scalar(
                        out=u[:, :w], in0=h2[:, c0 : c0 + w], scalar1=s2[:, :],
                        scalar2=t2[:, :], op0=A.mult, op1=A.add,
                    )
                    nc.vector.scalar_tensor_tensor(
                        out=a2[:, c0 : c0 + w], in0=u[:, :w], scalar=SLOPE,
                        in1=u[:, :w], op0=A.mult, op1=A.max,
                    )
            h3 = big.tile([128, WTB], f16, tag="hbuf")
            for ti, (c0, w) in enumerate(TILE_B):
                ps = psB.tile([128, 384], f32, tag="psB", name=f"h3p_{ti}")
                for u in range(2):
                    nc.tensor.matmul(
                        ps[64 * u : 64 * u + 64, :w],
                        l3[:, :],
                        a2[:, WTB * u + c0 : WTB * u + c0 + w],
                        start=True, stop=True,
                        tile_position=(0, 64 * u),
                    )
                copy_and_stats(3, ti, ps[:, :w], w, h3[:, c0 : c0 + w], ti % 2)
            for gi in range(GPC):
                cA = int(_OFF[gi])
                u = 0 if cA < WTB else 1
                diag_stats(3, gi, h3, cA - WTB * u, 64 * u, 64)

            s3, t3v = barrier(3, NTB, p8, 4, 5)

            # ================= PASS 4: apply3, mm4, h4 =================
            h4 = big.tile([128, WTB], f16, tag="hbuf")
            for si, (c0, w) in enumerate(SLAB_B):
                at = atp.tile([128, 1536], f16, tag="at", name=f"a3_{si}")
                nc.scalar.activation(
                    out=at[:, :w], in_=h3[:, c0 : c0 + w],
                    func=AF.Lrelu, scale=s3[:, :], bias=t3v[:, :], alpha=SLOPE,
                )
                for z in range(0, w, 384):
                    wz = min(384, w - z)
                    ti = (c0 + z) // 384
                    ps = psB.tile([128, 384], f32, tag="psB", name=f"h4p_{ti}")
                    nc.tensor.matmul(
                        ps[:, :wz], l4[:, :], at[:, z : z + wz],
                        start=True, stop=True,
                    )
                    copy_and_stats(
                        4, ti, ps[:, :wz], wz, h4[:, c0 + z : c0 + z + wz], ti % 2
                    )
            for gi in range(GPC):
                cA = int(_OFF[gi])
                u = 0 if cA < WTB else 1
                diag_stats(4, gi, h4, cA - WTB * u, 64 * u, 64)

            s4, t4v = barrier(4, NTB, p8, 6, 7)

            # ================= PASS 5: apply4, mm5, out =================
            outb = outp.tile([128, WOUT], f32)
            for pi in range(NP5):
                ps5 = psB.tile([128, 384], f32, tag="psB", name=f"h5p_{pi}")
                for k in range(4):
                    ti = 4 * pi + k
                    if ti >= NTB:
                        nc.vector.memset(ps5[32 * k : 32 * k + 16, :], 0.0)
                        continue
                    c0, w = TILE_B[ti]
                    at = atp.tile([128, 1536], f16, tag="at", name=f"a4_{ti}")
                    if ti % 3 == 2:
                        u = dtp.tile([128, 1536], f16, tag="dt2", name=f"u4_{ti}")
                        nc.vector.tensor_scalar(
                            out=u[:, :w], in0=h4[:, c0 : c0 + w], scalar1=s4[:, :],
                            scalar2=t4v[:, :], op0=A.mult, op1=A.add,
                        )
                        nc.vector.scalar_tensor_tensor(
                            out=at[:, :w], in0=u[:, :w], scalar=SLOPE,
                            in1=u[:, :w], op0=A.mult, op1=A.max,
                        )
                    else:
                        nc.scalar.activation(
                            out=at[:, :w], in_=h4[:, c0 : c0 + w],
                            func=AF.Lrelu, scale=s4[:, :], bias=t4v[:, :], alpha=SLOPE,
                        )
                    nc.tensor.matmul(
                        ps5[32 * k : 32 * k + 16, :w], l5[:, :], at[:, :w],
                        start=True, stop=True,
                        tile_position=(0, 32 * k),
                    )
                    if w < 384:
                        nc.vector.memset(ps5[32 * k : 32 * k + 16, w:384], 0.0)
                nc.scalar.activation(
                    out=outb[:, 384 * pi : 384 * pi + 384], in_=ps5[:, :],
                    func=AF.Identity, bias=b5b[:, :], scale=1.0,
                )
            nc.sync.dma_start(out_e[:, :], outb[:, :])

    nc.compile()
    return nc


def _host_inputs(x, W1, W2, W3, W4, W5, g1, be1, g2, be2, g3, be3, g4, be4, b5):
    xT = x.T.astype(np.float32)  # [64, 1536]

    lhsT1 = np.zeros((128, 32), np.float32)
    for d in range(2):
        lhsT1[64 * d : 64 * d + 64, 16 * d : 16 * d + 16] = W1.T
    lhsT2 = np.zeros((128, 128), np.float32)
    for r in range(8):
        lhsT2[16 * r : 16 * r + 16, 16 * r : 16 * r + 16] = W2.T
    lhsT3 = np.zeros((128, 64), np.float32)
    for r in range(8):
        lhsT3[16 * r : 16 * r + 16, 8 * r : 8 * r + 8] = W3.T
    lhsT4 = np.zeros((128, 128), np.float32)
    for b in range(16):
        lhsT4[8 * b : 8 * b + 8, 8 * b : 8 * b + 8] = W4.T
    lhsT5 = np.zeros((128, 16), np.float32)
    for b in range(16):
        lhsT5[8 * b : 8 * b + 8, b] = W5[0, :]

    q = np.arange(128)
    pat16 = (q[:, None] % 16 == q[None, :] % 16).astype(np.float32)
    pat8 = (q[:, None] % 8 == q[None, :] % 8).astype(np.float32)
    gb = np.stack(
        [
            g1[q % 16], be1[q % 16], g2[q % 16], be2[q % 16],
            g3[q % 8], be3[q % 8], g4[q % 8], be4[q % 8],
        ],
        axis=1,
    ).astype(np.float32)
    b5b = np.full((128, 1), float(b5[0]), np.float32)

    common = {
        "lhsT1": lhsT1.astype(np.float16),
        "lhsT2": lhsT2.astype(np.float16),
        "lhsT3": lhsT3.astype(np.float16),
        "lhsT4": lhsT4.astype(np.float16),
        "lhsT5": lhsT5.astype(np.float16),
        "pat16": pat16,
        "pat8": pat8,
        "gb": gb,
        "b5b": b5b,
    }

    in_maps = []
    for core in range(NC_):
        gl = _glist(core)
        xw = np.empty((64, WTA), np.float32)
        xp = np.zeros((128, 96), np.float32)
        for gi, g in enumerate(gl):
            o0, o1 = int(_OFF[gi]), int(_OFF[gi + 1])
            cols = (8 * g + np.arange(o1 - o0)) % N
            xw[:, o0:o1] = xT[:, cols]
            for pp in range(4):
                for d in range(2):
                    xp[64 * d : 64 * d + 64, 4 * gi + pp] = x[8 * g + 2 * pp + d, :]
        m = dict(common)
        m["xw"] = np.concatenate([xw, xw], axis=0).astype(np.float16)
        m["xp"] = xp
        m["xpn"] = -xp
        in_maps.append(m)
    return in_maps


def _decode_maps():
    """Static scatter maps: (core, partition, outcol) -> (row, col) of out[N,N]."""
    if "maps" in _CACHE:
        return _CACHE["maps"]
    rows = np.zeros((NC_, 128, WOUT), np.int32)
    cols = np.zeros((NC_, 128, WOUT), np.int32)
    valid = np.zeros((NC_, 128, WOUT), bool)
    for core in range(NC_):
        gl = _glist(core)
        for ti, (cb, w) in enumerate(TILE_B):
            pi, k = ti // 4, ti % 4
            for u in range(2):
                cA0 = WTB * u + cb
                for gi in range(GPC):
                    lo = max(int(_OFF[gi]), cA0)
                    hi = min(int(_OFF[gi + 1]), cA0 + w)
                    if lo >= hi:
                        continue
                    g = gl[gi]
                    jj = np.arange(lo, hi)
                    j = (8 * g + (jj - int(_OFF[gi]))) % N
                    oc = 384 * pi + (jj - cA0)
                    for r in range(8):
                        p = 32 * k + 8 * u + r
                        rows[core, p, oc] = 8 * g + r
                        cols[core, p, oc] = j
                        valid[core, p, oc] = True
    _CACHE["maps"] = (rows, cols, valid)
    return _CACHE["maps"]


def kernel(**inputs):
    global LAST_EXEC_NS
    import os

    x = np.asarray(inputs["x"], np.float32)
    args = [
        np.asarray(inputs[k], np.float32)
        for k in ("W1", "W2", "W3", "W4", "W5", "g1", "be1", "g2", "be2",
                  "g3", "be3", "g4", "be4", "b5")
    ]
    in_maps = _host_inputs(x, *args)

    if "nc" not in _CACHE:
        _CACHE["nc"] = _build()
    nc = _CACHE["nc"]

    trace = os.environ.get("KERNEL_TRACE", "0") == "1"
    res = run_bass_kernel_spmd(nc, in_maps, core_ids=list(range(NC_)), trace=trace)
    LAST_EXEC_NS = res.exec_time_ns

    rows, cols, valid = _decode_maps()
    out = np.zeros((N, N), np.float32)
    for core in range(NC_):
        raw = np.asarray(res.results[core]["out"])
        v = valid[core]
        out[rows[core][v], cols[core][v]] = raw[v]
    # mirror the uncovered orientations (covered set: every unordered pair once)
    if "mirror" not in _CACHE:
        cov = np.zeros((N, N), bool)
        for core in range(NC_):
            v = valid[core]
            cov[rows[core][v], cols[core][v]] = True
        _CACHE["mirror"] = ~cov
    m = _CACHE["mirror"]
    out[m] = out.T[m]
    return out


# revision 10
# speedup vs baseline: 1.4988x; 1.1080x over previous
"""Trainium2 Bass kernel for nn_AdjCompute (pairwise |x_i-x_j| -> 4x(1x1 conv+BN+lrelu) -> 1x1 conv).

v2: wrapped-band symmetric version. out[i,j] == out[j,i], so each 8-row group
g computes only a cyclic column window [8g, 8g + W_g) mod 1536 with
W_g = 776 for g < 96 and 768 for g >= 96. This covers every off-diagonal
8x8 block pair exactly once and every diagonal block fully; the mirror half
is assembled on the host. BN statistics use
  S_full = 2*S_computed - S_diagblocks.
All cores get identical op shapes (12 groups of each width class); per-core
variation (which rows, which wrapped columns) is carried entirely by input
data (xw = pre-gathered wrapped x columns, xp = pair scalars).

Device layout (per core, 24 groups, total computed cols WTA = 18528):
  stage A (64->16->16 ch): flat column stream; group gi at stream cols
    [OFF[gi], OFF[gi]+W), partition = 16*r + o (row-in-group, channel).
  stage B (16->8->8->1 ch): stream halves stacked: partition = 64*u + 8*r + o,
    stage-B col c in [0, 9264): u=0 <-> stage-A col c, u=1 <-> 9264 + c.
Output: raw [128, 2688] f32 stage-B stream dump per core; host unscrambles
and mirrors.
"""

import numpy as np

from concourse import bacc, mybir, tile
from concourse.bass_utils import run_bass_kernel_spmd

NC_ = 8
N = 1536
NTOT = float(N * N)
EPS = 1e-5
SLOPE = 0.01
GPC = 24  # groups per core

f32, f16 = mybir.dt.float32, mybir.dt.float16
A = mybir.AluOpType
AF = mybir.ActivationFunctionType

_CACHE = {}
LAST_EXEC_NS = None


def _glist(core):
    gl = []
    for t in range(12):
        gl.append(core + 8 * t)  # W = 776
        gl.append(96 + core + 8 * t)  # W = 768
    return gl


_LL = [776 if i % 2 == 0 else 768 for i in range(GPC)]  # identical for all cores
_OFF = np.concatenate([[0], np.cumsum(_LL)]).astype(int)
WTA = int(_OFF[-1])  # 18528
WTB = WTA // 2  # 9264
assert int(_OFF[12]) == WTB

# stage-A per-group tiling (chunks of <=512, one PSUM bank) for mm1/copy/stats
TILE_A = []  # (gi, stream_start, width)
for gi in range(GPC):
    L = _LL[gi]
    c = 0
    while c < L:
        w = min(512, L - c)
        TILE_A.append((gi, int(_OFF[gi]) + c, w))
        c += w
NTA = len(TILE_A)  # 48

# flat stage-A tiling for mm2 (512 chunks)
TILE_F = []
c = 0
while c < WTA:
    w = min(512, WTA - c)
    TILE_F.append((c, w))
    c += w
NTF = len(TILE_F)  # 37

SLAB_A = []
c = 0
while c < WTA:
    w = min(1536, WTA - c)
    SLAB_A.append((c, w))
    c += w

# stage-B tiling (384 chunks)
TILE_B = []
c = 0
while c < WTB:
    w = min(384, WTB - c)
    TILE_B.append((c, w))
    c += w
NTB = len(TILE_B)  # 25
NP5 = (NTB + 3) // 4  # 7 psum5 tiles
WOUT = NP5 * 384  # 2688

SLAB_B = []
c = 0
while c < WTB:
    w = min(1536, WTB - c)
    SLAB_B.append((c, w))
    c += w


def _build():
    nc = bacc.Bacc("TRN2", target_bir_lowering=False, debug=False, num_devices=NC_)

    def din(name, shape, dt):
        return nc.dram_tensor(name, shape, dt, kind="ExternalInput")

    xw_e = din("xw", [128, WTA], f16)
    xp_e = din("xp", [128, 96], f32)
    l1_e = din("lhsT1", [128, 32], f16)
    l1n_e = din("lhsT1n", [128, 32], f16)
    l2_e = din("lhsT2", [128, 128], f16)
    l3_e = din("lhsT3", [128, 64], f16)
    l4_e = din("lhsT4", [128, 128], f16)
    l5_e = din("lhsT5", [128, 16], f16)
    p16_e = din("pat16", [128, 128], f32)
    p8_e = din("pat8", [128, 128], f32)
    gb_e = din("gb", [128, 8], f32)
    b5_e = din("b5b", [128, 1], f32)
    out_e = nc.dram_tensor("out", [128, WOUT], f32, kind="ExternalOutput")

    with tile.TileContext(nc) as tc:
        with (
            tc.tile_pool(name="const", bufs=1) as cpool,
            tc.tile_pool(name="big", bufs=3) as big,
            tc.tile_pool(name="adjp", bufs=4) as adjp,
            tc.tile_pool(name="dtp", bufs=2) as dtp,
            tc.tile_pool(name="atp", bufs=3) as atp,
            tc.tile_pool(name="jkp", bufs=2) as jkp,
            tc.tile_pool(name="statp", bufs=1) as statp,
            tc.tile_pool(name="smallp", bufs=1) as smallp,
            tc.tile_pool(name="outp", bufs=1) as outp,
            tc.tile_pool(name="psA", bufs=4, space="PSUM") as psA,
            tc.tile_pool(name="psB", bufs=3, space="PSUM") as psB,
            tc.tile_pool(name="psS", bufs=1, space="PSUM") as psS,
            tc.tile_pool(name="dram", bufs=1, space="DRAM") as dram,
        ):
            # ---- constants ----
            xp = cpool.tile([128, 96], f32)
            l1 = cpool.tile([128, 32], f16)
            l1n = cpool.tile([128, 32], f16)
            l2 = cpool.tile([128, 128], f16)
            l3 = cpool.tile([128, 64], f16)
            l4 = cpool.tile([128, 128], f16)
            l5 = cpool.tile([128, 16], f16)
            p16 = cpool.tile([128, 128], f32)
            p8 = cpool.tile([128, 128], f32)
            gb = cpool.tile([128, 8], f32)
            b5b = cpool.tile([128, 1], f32)
            for t, e in [
                (xp, xp_e), (l1, l1_e), (l1n, l1n_e), (l2, l2_e),
                (l3, l3_e), (l4, l4_e), (l5, l5_e), (p16, p16_e), (p8, p8_e),
                (gb, gb_e), (b5b, b5_e),
            ]:
                sl = (slice(None),) * len(t.shape)
                nc.sync.dma_start(t[sl], e[sl])

            # wrapped x-column stream (big pool slot, freed after pass 1)
            xw = big.tile([128, WTA], f16, tag="hbuf")
            for gi in range(GPC):
                o0, o1 = int(_OFF[gi]), int(_OFF[gi + 1])
                nc.sync.dma_start(xw[:, o0:o1], xw_e[:, o0:o1])

            h1 = big.tile([128, WTA], f16, tag="hbuf")

            sumb = {}
            sqb = {}
            dsb = {}
            dqb = {}
            stbn = {}
            n_bn = {}
            n_s2 = {}
            for k, nt in [(1, NTA), (2, NTF), (3, NTB), (4, NTB)]:
                sumb[k] = statp.tile([128, nt], f32, name=f"sumb{k}")
                sqb[k] = statp.tile([128, nt], f32, name=f"sqb{k}")
                stbn[k] = statp.tile([128, 6 * nt], f32, name=f"stbn{k}")
                dsb[k] = statp.tile([128, 4], f32, name=f"dsb{k}")
                dqb[k] = statp.tile([128, 4], f32, name=f"dqb{k}")
                nc.vector.memset(dsb[k][:, :], 0.0)
                nc.vector.memset(dqb[k][:, :], 0.0)
                n_bn[k] = 0
                n_s2[k] = 0
            w_bn = {1: 0, 2: 0, 3: 0, 4: 0}

            def copy_and_stats(k, ti, ps, wid, dst, eng):
                # style 0: ACT plain copy + DVE bn_stats (no accumulators)
                # style 1: DVE copy-with-sum-accum + ACT square-with-accum
                if eng == 0:
                    nc.scalar.activation(out=dst, in_=ps, func=AF.Copy)
                    j = n_bn[k]
                    n_bn[k] += 1
                    w_bn[k] += wid
                    nc.vector.bn_stats(stbn[k][:, 6 * j : 6 * j + 6], ps)
                else:
                    j = n_s2[k]
                    n_s2[k] += 1
                    nc.vector.tensor_scalar(
                        out=dst, in0=ps, scalar1=1.0, scalar2=0.0,
                        op0=A.mult, op1=A.add,
                        accum_out=sumb[k][:, j : j + 1],
                    )
                    jk = jkp.tile([128, 512], f16, tag="jk", name=f"jk{k}_{ti}")
                    nc.scalar.activation(
                        out=jk[:, :wid], in_=ps, func=AF.Square,
                        accum_out=sqb[k][:, j : j + 1],
                    )

            def diag_stats_batched(k, hst, stage):
                # diag blocks of group gi start at stream col OFF[gi]:
                # {1544*t, 1544*t + 776} = 8*(193*t + {0, 97}).
                # stage A: 12 t-blocks over full 128 partitions;
                # stage B: 6 t-blocks per u-half (u=0: partitions 0:64, u=1: 64:128).
                if stage == 0:
                    nt = 12
                    view = hst.rearrange("p (t q j) -> p t q j", t=nt, q=193, j=8)
                    parts = [(0, 128)]
                else:
                    nt = 6
                    view = hst.rearrange("p (t q j) -> p t q j", t=nt, q=193, j=8)
                    parts = [(0, 64), (64, 64)]
                col = -1
                for p0, pn in parts:
                    for qi in (0, 97):
                        col += 1
                        jd = smallp.tile(
                            [128, 12, 8], f16, name=f"jd{k}_{col}_{p0}", tag="jd"
                        )
                        nc.vector.tensor_scalar(
                            out=jd[p0 : p0 + pn, :nt, :],
                            in0=view[p0 : p0 + pn, :, qi, :],
                            scalar1=1.0, scalar2=0.0, op0=A.mult, op1=A.add,
                            accum_out=dsb[k][p0 : p0 + pn, col : col + 1],
                        )
                        jd2 = smallp.tile(
                            [128, 12, 8], f16, name=f"jd2{k}_{col}_{p0}", tag="jd2"
                        )
                        nc.scalar.activation(
                            out=jd2[p0 : p0 + pn, :nt, :],
                            in_=view[p0 : p0 + pn, :, qi, :],
                            func=AF.Square,
                            accum_out=dqb[k][p0 : p0 + pn, col : col + 1],
                        )


            # ================= PASS 1 =================
            gi2tiles = {}
            for ti, (gi, c0, w) in enumerate(TILE_A):
                gi2tiles.setdefault(gi, []).append((ti, c0, w))

            for gi in range(GPC):
                L = _LL[gi]
                o0 = int(_OFF[gi])
                relus = []
                mins = []
                for pp in range(4):
                    idx = 4 * gi + pp
                    rl = adjp.tile([128, 776], f16, tag="adj", name=f"rl_{idx}")
                    nc.vector.tensor_scalar(
                        out=rl[:, :L], in0=xw[:, o0 : o0 + L],
                        scalar1=xp[:, idx : idx + 1], scalar2=0.0,
                        op0=A.subtract, op1=A.max,
                    )
                    mn = dtp.tile([128, 776], f16, tag="dt", name=f"mn_{idx}")
                    nc.vector.tensor_scalar(
                        out=mn[:, :L], in0=xw[:, o0 : o0 + L],
                        scalar1=xp[:, idx : idx + 1], scalar2=0.0,
                        op0=A.subtract, op1=A.min,
                    )
                    relus.append(rl)
                    mins.append(mn)
                for ti, c0, w in gi2tiles[gi]:
                    lc = c0 - o0
                    ps = psA.tile([128, 512], f32, tag="psA", name=f"h1p_{ti}")
                    for pp in range(4):
                        nc.tensor.matmul(
                            ps[32 * pp : 32 * pp + 32, :w],
                            l1[:, :],
                            relus[pp][:, lc : lc + w],
                            start=True, stop=False,
                            tile_position=(0, 32 * pp),
                        )
                        nc.tensor.matmul(
                            ps[32 * pp : 32 * pp + 32, :w],
                            l1n[:, :],
                            mins[pp][:, lc : lc + w],
                            start=False, stop=True,
                            tile_position=(0, 32 * pp),
                        )
                    copy_and_stats(1, ti, ps[:, :w], w, h1[:, c0 : c0 + w], ti % 3 == 2)
            diag_stats_batched(1, h1, 0)

            # ---- barrier ----
            def barrier(k, pat, gcol, becol):
                nbn, ns2, wbn = n_bn[k], n_s2[k], w_bn[k]
                ba = smallp.tile([128, 2], f32, name=f"ba{k}")
                nc.vector.bn_aggr(ba[:, :], stbn[k][:, : 6 * nbn])
                m2 = smallp.tile([128, 1], f32, name=f"m2_{k}")
                nc.scalar.activation(out=m2[:, :], in_=ba[:, 0:1], func=AF.Square)
                q1 = smallp.tile([128, 1], f32, name=f"q1_{k}")
                nc.vector.tensor_tensor(
                    out=q1[:, :], in0=ba[:, 1:2], in1=m2[:, :], op=A.add,
                )
                s2r = smallp.tile([128, 1], f32, name=f"s2r{k}")
                nc.vector.tensor_reduce(
                    out=s2r[:, :], in_=sumb[k][:, :ns2],
                    axis=mybir.AxisListType.X, op=A.add,
                )
                q2r = smallp.tile([128, 1], f32, name=f"q2r{k}")
                nc.vector.tensor_reduce(
                    out=q2r[:, :], in_=sqb[k][:, :ns2],
                    axis=mybir.AxisListType.X, op=A.add,
                )
                ds_l = smallp.tile([128, 1], f32, name=f"dsl{k}")
                nc.vector.tensor_reduce(
                    out=ds_l[:, :], in_=dsb[k][:, :],
                    axis=mybir.AxisListType.X, op=A.add,
                )
                dq_l = smallp.tile([128, 1], f32, name=f"dql{k}")
                nc.vector.tensor_reduce(
                    out=dq_l[:, :], in_=dqb[k][:, :],
                    axis=mybir.AxisListType.X, op=A.add,
                )
                s_t = smallp.tile([128, 1], f32, name=f"st{k}")
                nc.vector.scalar_tensor_tensor(
                    out=s_t[:, :], in0=ba[:, 0:1], scalar=float(wbn), in1=s2r[:, :],
                    op0=A.mult, op1=A.add,
                )
                q_t = smallp.tile([128, 1], f32, name=f"qt{k}")
                nc.vector.scalar_tensor_tensor(
                    out=q_t[:, :], in0=q1[:, :], scalar=float(wbn), in1=q2r[:, :],
                    op0=A.mult, op1=A.add,
                )
                # (2*S - DS)/2 = S - DS/2 ; the 2/NTOT scale is applied post-AG
                sq = smallp.tile([128, 2], f32, name=f"sq{k}")
                nc.vector.scalar_tensor_tensor(
                    out=sq[:, 0:1], in0=ds_l[:, :], scalar=-0.5, in1=s_t[:, :],
                    op0=A.mult, op1=A.add,
                )
                nc.vector.scalar_tensor_tensor(
                    out=sq[:, 1:2], in0=dq_l[:, :], scalar=-0.5, in1=q_t[:, :],
                    op0=A.mult, op1=A.add,
                )
                pf = psS.tile([128, 2], f32, tag="psS", name=f"pf{k}")
                nc.tensor.matmul(pf[:, :], pat[:, :], sq[:, :], start=True, stop=True)
                gl = smallp.tile([128, 2], f32, name=f"gl{k}")
                nc.scalar.activation(out=gl[:, :], in_=pf[:, :], func=AF.Copy)
                agi = dram.tile([128, 2], f32, name=f"agi{k}")
                ago = dram.tile([128 * NC_, 2], f32, addr_space="Shared", name=f"ago{k}")
                nc.sync.dma_start(agi[:, :], gl[:, :])
                nc.gpsimd.collective_compute(
                    "AllGather", A.bypass,
                    replica_groups=[list(range(NC_))],
                    ins=[agi.opt()], outs=[ago.opt()],
                )
                agv = smallp.tile([128, 2, NC_], f32, name=f"agv{k}")
                nc.sync.dma_start(
                    agv[:, :, :], ago.rearrange("(b p) c -> p c b", b=NC_),
                )
                gt = smallp.tile([128, 2], f32, name=f"gt{k}")
                nc.vector.tensor_reduce(
                    out=gt[:, :], in_=agv[:, :, :],
                    axis=mybir.AxisListType.X, op=A.add,
                )
                negmean = smallp.tile([128, 1], f32, name=f"nm{k}")
                nc.vector.tensor_scalar(
                    out=negmean[:, :], in0=gt[:, 0:1], scalar1=-2.0 / NTOT,
                    scalar2=None, op0=A.mult,
                )
                ex2e = smallp.tile([128, 1], f32, name=f"ex{k}")
                nc.vector.tensor_scalar(
                    out=ex2e[:, :], in0=gt[:, 1:2], scalar1=2.0 / NTOT,
                    scalar2=EPS, op0=A.mult, op1=A.add,
                )
                msq = smallp.tile([128, 1], f32, name=f"ms{k}")
                nc.scalar.activation(out=msq[:, :], in_=negmean[:, :], func=AF.Square)
                vpe = smallp.tile([128, 1], f32, name=f"vp{k}")
                nc.vector.scalar_tensor_tensor(
                    out=vpe[:, :], in0=msq[:, :], scalar=-1.0, in1=ex2e[:, :],
                    op0=A.mult, op1=A.add,
                )
                rinv = smallp.tile([128, 1], f32, name=f"ri{k}")
                nc.vector.reciprocal(rinv[:, :], vpe[:, :])
                rstd = smallp.tile([128, 1], f32, name=f"rs{k}")
                nc.scalar.activation(out=rstd[:, :], in_=rinv[:, :], func=AF.Sqrt)
                sk = smallp.tile([128, 1], f32, name=f"s{k}")
                nc.vector.tensor_tensor(
                    out=sk[:, :], in0=rstd[:, :], in1=gb[:, gcol : gcol + 1], op=A.mult,
                )
                tk = smallp.tile([128, 1], f32, name=f"t{k}")
                nc.vector.scalar_tensor_tensor(
                    out=tk[:, :], in0=sk[:, :], scalar=negmean[:, :],
                    in1=gb[:, becol : becol + 1], op0=A.mult, op1=A.add,
                )
                return sk, tk

            s1, t1 = barrier(1, p16, 0, 1)

            # ================= PASS 2: apply1, mm2, h2 =================
            h2 = big.tile([128, WTA], f16, tag="hbuf")
            for si, (c0, w) in enumerate(SLAB_A):
                at = atp.tile([128, 1536], f16, tag="at", name=f"a1_{si}")
                nc.scalar.activation(
                    out=at[:, :w], in_=h1[:, c0 : c0 + w],
                    func=AF.Lrelu, scale=s1[:, :], bias=t1[:, :], alpha=SLOPE,
                )
                for z in range(0, w, 512):
                    wz = min(512, w - z)
                    ti = (c0 + z) // 512
                    ps = psA.tile([128, 512], f32, tag="psA", name=f"h2p_{ti}")
                    nc.tensor.matmul(
                        ps[:, :wz], l2[:, :], at[:, z : z + wz],
                        start=True, stop=True,
                    )
                    copy_and_stats(
                        2, ti, ps[:, :wz], wz, h2[:, c0 + z : c0 + z + wz], ti % 3 == 2
                    )
            diag_stats_batched(2, h2, 0)

            s2, t2 = barrier(2, p16, 2, 3)

            # ================= PASS 3: apply2, mm3, h3 =================
            a2 = big.tile([128, WTA], f16, tag="hbuf")
            for si, (c0, w) in enumerate(SLAB_A):
                if si % 2 == 0:
                    nc.scalar.activation(
                        out=a2[:, c0 : c0 + w], in_=h2[:, c0 : c0 + w],
                        func=AF.Lrelu, scale=s2[:, :], bias=t2[:, :], alpha=SLOPE,
                    )
                else:
                    u = dtp.tile([128, 1536], f16, tag="dt2", name=f"u2_{si}")
                    nc.vector.tensor_scalar(
                        out=u[:, :w], in0=h2[:, c0 : c0 + w], scalar1=s2[:, :],
                        scalar2=t2[:, :], op0=A.mult, op1=A.add,
                    )
                    nc.vector.scalar_tensor_tensor(
                        out=a2[:, c0 : c0 + w], in0=u[:, :w], scalar=SLOPE,
                        in1=u[:, :w], op0=A.mult, op1=A.max,
                    )
            h3 = big.tile([128, WTB], f16, tag="hbuf")
            for ti, (c0, w) in enumerate(TILE_B):
                ps = psB.tile([128, 384], f32, tag="psB", name=f"h3p_{ti}")
                for u in range(2):
                    nc.tensor.matmul(
                        ps[64 * u : 64 * u + 64, :w],
                        l3[:, :],
                        a2[:, WTB * u + c0 : WTB * u + c0 + w],
                        start=True, stop=True,
                        tile_position=(0, 64 * u),
                    )
                copy_and_stats(3, ti, ps[:, :w], w, h3[:, c0 : c0 + w], ti % 3 == 2)
            diag_stats_batched(3, h3, 1)

            s3, t3v = barrier(3, p8, 4, 5)

            # ================= PASS 4: apply3, mm4, h4 =================
            h4 = big.tile([128, WTB], f16, tag="hbuf")
            for si, (c0, w) in enumerate(SLAB_B):
                at = atp.tile([128, 1536], f16, tag="at", name=f"a3_{si}")
                nc.scalar.activation(
                    out=at[:, :w], in_=h3[:, c0 : c0 + w],
                    func=AF.Lrelu, scale=s3[:, :], bias=t3v[:, :], alpha=SLOPE,
                )
                for z in range(0, w, 384):
                    wz = min(384, w - z)
                    ti = (c0 + z) // 384
                    ps = psB.tile([128, 384], f32, tag="psB", name=f"h4p_{ti}")
                    nc.tensor.matmul(
                        ps[:, :wz], l4[:, :], at[:, z : z + wz],
                        start=True, stop=True,
                    )
                    copy_and_stats(
                        4, ti, ps[:, :wz], wz, h4[:, c0 + z : c0 + z + wz], ti % 3 == 2
                    )
            diag_stats_batched(4, h4, 1)

            s4, t4v = barrier(4, p8, 6, 7)

            # ================= PASS 5: apply4, mm5, out =================
            outb = outp.tile([128, WOUT], f32)
            for pi in range(NP5):
                ps5 = psB.tile([128, 384], f32, tag="psB", name=f"h5p_{pi}")
                for k in range(4):
                    ti = 4 * pi + k
                    if ti >= NTB:
                        nc.vector.memset(ps5[32 * k : 32 * k + 16, :], 0.0)
                        continue
                    c0, w = TILE_B[ti]
                    at = atp.tile([128, 1536], f16, tag="at", name=f"a4_{ti}")
                    if ti % 3 == 2:
                        u = dtp.tile([128, 1536], f16, tag="dt2", name=f"u4_{ti}")
                        nc.vector.tensor_scalar(
                            out=u[:, :w], in0=h4[:, c0 : c0 + w], scalar1=s4[:, :],
                            scalar2=t4v[:, :], op0=A.mult, op1=A.add,
                        )
                        nc.vector.scalar_tensor_tensor(
                            out=at[:, :w], in0=u[:, :w], scalar=SLOPE,
                            in1=u[:, :w], op0=A.mult, op1=A.max,
                        )
                    else:
                        nc.scalar.activation(
                            out=at[:, :w], in_=h4[:, c0 : c0 + w],
                            func=AF.Lrelu, scale=s4[:, :], bias=t4v[:, :], alpha=SLOPE,
                        )
                    nc.tensor.matmul(
                        ps5[32 * k : 32 * k + 16, :w], l5[:, :], at[:, :w],
                        start=True, stop=True,
                        tile_position=(0, 32 * k),
                    )
                    if w < 384:
                        nc.vector.memset(ps5[32 * k : 32 * k + 16, w:384], 0.0)
                nc.scalar.activation(
                    out=outb[:, 384 * pi : 384 * pi + 384], in_=ps5[:, :],
                    func=AF.Identity, bias=b5b[:, :], scale=1.0,
                )
            nc.sync.dma_start(out_e[:, :], outb[:, :])

    nc.compile()
    return nc


def _host_inputs(x, W1, W2, W3, W4, W5, g1, be1, g2, be2, g3, be3, g4, be4, b5):
    xT = x.T.astype(np.float32)  # [64, 1536]

    lhsT1 = np.zeros((128, 32), np.float32)
    for d in range(2):
        lhsT1[64 * d : 64 * d + 64, 16 * d : 16 * d + 16] = W1.T
    lhsT2 = np.zeros((128, 128), np.float32)
    for r in range(8):
        lhsT2[16 * r : 16 * r + 16, 16 * r : 16 * r + 16] = W2.T
    lhsT3 = np.zeros((128, 64), np.float32)
    for r in range(8):
        lhsT3[16 * r : 16 * r + 16, 8 * r : 8 * r + 8] = W3.T
    lhsT4 = np.zeros((128, 128), np.float32)
    for b in range(16):
        lhsT4[8 * b : 8 * b + 8, 8 * b : 8 * b + 8] = W4.T
    lhsT5 = np.zeros((128, 16), np.float32)
    for b in range(16):
        lhsT5[8 * b : 8 * b + 8, b] = W5[0, :]

    q = np.arange(128)
    pat16 = (q[:, None] % 16 == q[None, :] % 16).astype(np.float32)
    pat8 = (q[:, None] % 8 == q[None, :] % 8).astype(np.float32)
    gb = np.stack(
        [
            g1[q % 16], be1[q % 16], g2[q % 16], be2[q % 16],
            g3[q % 8], be3[q % 8], g4[q % 8], be4[q % 8],
        ],
        axis=1,
    ).astype(np.float32)
    b5b = np.full((128, 1), float(b5[0]), np.float32)

    common = {
        "lhsT1": lhsT1.astype(np.float16),
        "lhsT1n": (-lhsT1).astype(np.float16),
        "lhsT2": lhsT2.astype(np.float16),
        "lhsT3": lhsT3.astype(np.float16),
        "lhsT4": lhsT4.astype(np.float16),
        "lhsT5": lhsT5.astype(np.float16),
        "pat16": pat16,
        "pat8": pat8,
        "gb": gb,
        "b5b": b5b,
    }

    in_maps = []
    for core in range(NC_):
        gl = _glist(core)
        xw = np.empty((64, WTA), np.float32)
        xp = np.zeros((128, 96), np.float32)
        for gi, g in enumerate(gl):
            o0, o1 = int(_OFF[gi]), int(_OFF[gi + 1])
            cols = (8 * g + np.arange(o1 - o0)) % N
            xw[:, o0:o1] = xT[:, cols]
            for pp in range(4):
                for d in range(2):
                    xp[64 * d : 64 * d + 64, 4 * gi + pp] = x[8 * g + 2 * pp + d, :]
        m = dict(common)
        m["xw"] = np.concatenate([xw, xw], axis=0).astype(np.float16)
        m["xp"] = xp
        in_maps.append(m)
    return in_maps


def _decode_maps():
    """Static scatter maps: (core, partition, outcol) -> (row, col) of out[N,N]."""
    if "maps" in _CACHE:
        return _CACHE["maps"]
    rows = np.zeros((NC_, 128, WOUT), np.int32)
    cols = np.zeros((NC_, 128, WOUT), np.int32)
    valid = np.zeros((NC_, 128, WOUT), bool)
    for core in range(NC_):
        gl = _glist(core)
        for ti, (cb, w) in enumerate(TILE_B):
            pi, k = ti // 4, ti % 4
            for u in range(2):
                cA0 = WTB * u + cb
                for gi in range(GPC):
                    lo = max(int(_OFF[gi]), cA0)
                    hi = min(int(_OFF[gi + 1]), cA0 + w)
                    if lo >= hi:
                        continue
                    g = gl[gi]
                    jj = np.arange(lo, hi)
                    j = (8 * g + (jj - int(_OFF[gi]))) % N
                    oc = 384 * pi + (jj - cA0)
                    for r in range(8):
                        p = 32 * k + 8 * u + r
                        rows[core, p, oc] = 8 * g + r
                        cols[core, p, oc] = j
                        valid[core, p, oc] = True
    _CACHE["maps"] = (rows, cols, valid)
    return _CACHE["maps"]


def kernel(**inputs):
    global LAST_EXEC_NS
    import os

    x = np.asarray(inputs["x"], np.float32)
    args = [
        np.asarray(inputs[k], np.float32)
        for k in ("W1", "W2", "W3", "W4", "W5", "g1", "be1", "g2", "be2",
                  "g3", "be3", "g4", "be4", "b5")
    ]
    in_maps = _host_inputs(x, *args)

    if "nc" not in _CACHE:
        _CACHE["nc"] = _build()
    nc = _CACHE["nc"]

    trace = os.environ.get("KERNEL_TRACE", "0") == "1"
    res = run_bass_kernel_spmd(nc, in_maps, core_ids=list(range(NC_)), trace=trace)
    LAST_EXEC_NS = res.exec_time_ns

    rows, cols, valid = _decode_maps()
    out = np.zeros((N, N), np.float32)
    for core in range(NC_):
        raw = np.asarray(res.results[core]["out"])
        v = valid[core]
        out[rows[core][v], cols[core][v]] = raw[v]
    # mirror the uncovered orientations (covered set: every unordered pair once)
    if "mirror" not in _CACHE:
        cov = np.zeros((N, N), bool)
        for core in range(NC_):
            v = valid[core]
            cov[rows[core][v], cols[core][v]] = True
        _CACHE["mirror"] = ~cov
    m = _CACHE["mirror"]
    out[m] = out.T[m]
    return out


# revision 12
# speedup vs baseline: 1.5222x; 1.0156x over previous
"""Trainium2 Bass kernel for nn_AdjCompute (pairwise |x_i-x_j| -> 4x(1x1 conv+BN+lrelu) -> 1x1 conv).

v2: wrapped-band symmetric version. out[i,j] == out[j,i], so each 8-row group
g computes only a cyclic column window [8g, 8g + W_g) mod 1536 with
W_g = 776 for g < 96 and 768 for g >= 96. This covers every off-diagonal
8x8 block pair exactly once and every diagonal block fully; the mirror half
is assembled on the host. BN statistics use
  S_full = 2*S_computed - S_diagblocks.
All cores get identical op shapes (12 groups of each width class); per-core
variation (which rows, which wrapped columns) is carried entirely by input
data (xw = pre-gathered wrapped x columns, xp = pair scalars).

Device layout (per core, 24 groups, total computed cols WTA = 18528):
  stage A (64->16->16 ch): flat column stream; group gi at stream cols
    [OFF[gi], OFF[gi]+W), partition = 16*r + o (row-in-group, channel).
  stage B (16->8->8->1 ch): stream halves stacked: partition = 64*u + 8*r + o,
    stage-B col c in [0, 9264): u=0 <-> stage-A col c, u=1 <-> 9264 + c.
Output: raw [128, 2688] f32 stage-B stream dump per core; host unscrambles
and mirrors.
"""

import numpy as np

from concourse import bacc, mybir, tile
from concourse.bass_utils import run_bass_kernel_spmd

NC_ = 8
N = 1536
NTOT = float(N * N)
EPS = 1e-5
SLOPE = 0.01
GPC = 24  # groups per core

f32, f16 = mybir.dt.float32, mybir.dt.float16
A = mybir.AluOpType
AF = mybir.ActivationFunctionType

_CACHE = {}
LAST_EXEC_NS = None


def _glist(core):
    gl = []
    for t in range(12):
        gl.append(core + 8 * t)  # W = 776
        gl.append(96 + core + 8 * t)  # W = 768
    return gl


_LL = [776 if i % 2 == 0 else 768 for i in range(GPC)]  # identical for all cores
_OFF = np.concatenate([[0], np.cumsum(_LL)]).astype(int)
WTA = int(_OFF[-1])  # 18528
WTB = WTA // 2  # 9264
assert int(_OFF[12]) == WTB

# stage-A per-group tiling (chunks of <=512, one PSUM bank) for mm1/copy/stats
TILE_A = []  # (gi, stream_start, width)
for gi in range(GPC):
    L = _LL[gi]
    c = 0
    while c < L:
        w = min(512, L - c)
        TILE_A.append((gi, int(_OFF[gi]) + c, w))
        c += w
NTA = len(TILE_A)  # 48

# flat stage-A tiling for mm2 (512 chunks)
TILE_F = []
c = 0
while c < WTA:
    w = min(512, WTA - c)
    TILE_F.append((c, w))
    c += w
NTF = len(TILE_F)  # 37

SLAB_A = []
c = 0
while c < WTA:
    w = min(1536, WTA - c)
    SLAB_A.append((c, w))
    c += w

# stage-B tiling (384 chunks)
TILE_B = []
c = 0
while c < WTB:
    w = min(384, WTB - c)
    TILE_B.append((c, w))
    c += w
NTB = len(TILE_B)  # 25
NP5 = (NTB + 3) // 4  # 7 psum5 tiles
WOUT = NP5 * 384  # 2688

SLAB_B = []
c = 0
while c < WTB:
    w = min(1536, WTB - c)
    SLAB_B.append((c, w))
    c += w


def _build():
    nc = bacc.Bacc("TRN2", target_bir_lowering=False, debug=False, num_devices=NC_)

    def din(name, shape, dt):
        return nc.dram_tensor(name, shape, dt, kind="ExternalInput")

    xw_e = din("xw", [128, WTA], f16)
    xp_e = din("xp", [128, 96], f32)
    l1_e = din("lhsT1", [128, 32], f16)
    l1n_e = din("lhsT1n", [128, 32], f16)
    l2_e = din("lhsT2", [128, 128], f16)
    l3_e = din("lhsT3", [128, 64], f16)
    l4_e = din("lhsT4", [128, 128], f16)
    l5_e = din("lhsT5", [128, 16], f16)
    p16_e = din("pat16", [128, 128], f32)
    p8_e = din("pat8", [128, 128], f32)
    gb_e = din("gb", [128, 8], f32)
    b5_e = din("b5b", [128, 1], f32)
    out_e = nc.dram_tensor("out", [128, WOUT], f32, kind="ExternalOutput")

    with tile.TileContext(nc) as tc:
        with (
            tc.tile_pool(name="const", bufs=1) as cpool,
            tc.tile_pool(name="big", bufs=3) as big,
            tc.tile_pool(name="adjp", bufs=4) as adjp,
            tc.tile_pool(name="dtp", bufs=2) as dtp,
            tc.tile_pool(name="atp", bufs=3) as atp,
            tc.tile_pool(name="jkp", bufs=2) as jkp,
            tc.tile_pool(name="statp", bufs=1) as statp,
            tc.tile_pool(name="smallp", bufs=1) as smallp,
            tc.tile_pool(name="outp", bufs=1) as outp,
            tc.tile_pool(name="psA", bufs=4, space="PSUM") as psA,
            tc.tile_pool(name="psB", bufs=3, space="PSUM") as psB,
            tc.tile_pool(name="psS", bufs=1, space="PSUM") as psS,
            tc.tile_pool(name="dram", bufs=1, space="DRAM") as dram,
        ):
            # ---- constants ----
            xp = cpool.tile([128, 96], f32)
            l1 = cpool.tile([128, 32], f16)
            l1n = cpool.tile([128, 32], f16)
            l2 = cpool.tile([128, 128], f16)
            l3 = cpool.tile([128, 64], f16)
            l4 = cpool.tile([128, 128], f16)
            l5 = cpool.tile([128, 16], f16)
            p16 = cpool.tile([128, 128], f32)
            p8 = cpool.tile([128, 128], f32)
            gb = cpool.tile([128, 8], f32)
            b5b = cpool.tile([128, 1], f32)
            for t, e in [
                (xp, xp_e), (l1, l1_e), (l1n, l1n_e), (l2, l2_e),
                (l3, l3_e), (l4, l4_e), (l5, l5_e), (p16, p16_e), (p8, p8_e),
                (gb, gb_e), (b5b, b5_e),
            ]:
                sl = (slice(None),) * len(t.shape)
                nc.sync.dma_start(t[sl], e[sl])

            # wrapped x-column stream (big pool slot, freed after pass 1)
            xw = big.tile([128, WTA], f16, tag="hbuf")
            for gi in range(GPC):
                o0, o1 = int(_OFF[gi]), int(_OFF[gi + 1])
                nc.sync.dma_start(xw[:, o0:o1], xw_e[:, o0:o1])

            h1 = big.tile([128, WTA], f16, tag="hbuf")

            sumb = {}
            sqb = {}
            dsb = {}
            dqb = {}
            stbn = {}
            n_bn = {}
            n_s2 = {}
            for k, nt in [(1, NTA), (2, NTF), (3, NTB), (4, NTB)]:
                sumb[k] = statp.tile([128, nt], f32, name=f"sumb{k}")
                sqb[k] = statp.tile([128, nt], f32, name=f"sqb{k}")
                stbn[k] = statp.tile([128, 6 * nt], f32, name=f"stbn{k}")
                dsb[k] = statp.tile([128, 4], f32, name=f"dsb{k}")
                dqb[k] = statp.tile([128, 4], f32, name=f"dqb{k}")
                nc.vector.memset(dsb[k][:, :], 0.0)
                nc.vector.memset(dqb[k][:, :], 0.0)
                n_bn[k] = 0
                n_s2[k] = 0
            w_bn = {1: 0, 2: 0, 3: 0, 4: 0}

            def copy_and_stats(k, ti, ps, wid, dst, eng):
                nc.scalar.activation(out=dst, in_=ps, func=AF.Copy)
                j = n_bn[k]
                n_bn[k] += 1
                w_bn[k] += wid
                nc.vector.bn_stats(stbn[k][:, 6 * j : 6 * j + 6], dst)

            def diag_stats_batched(k, hst, stage):
                # diag blocks of group gi start at stream col OFF[gi]:
                # {1544*t, 1544*t + 776} = 8*(193*t + {0, 97}).
                # stage A: 12 t-blocks over full 128 partitions;
                # stage B: 6 t-blocks per u-half (u=0: partitions 0:64, u=1: 64:128).
                if stage == 0:
                    nt = 12
                    view = hst.rearrange("p (t q j) -> p t q j", t=nt, q=193, j=8)
                    parts = [(0, 128)]
                else:
                    nt = 6
                    view = hst.rearrange("p (t q j) -> p t q j", t=nt, q=193, j=8)
                    parts = [(0, 64), (64, 64)]
                col = -1
                for p0, pn in parts:
                    for qi in (0, 97):
                        col += 1
                        jd = smallp.tile(
                            [128, 12, 8], f16, name=f"jd{k}_{col}_{p0}", tag="jd"
                        )
                        nc.vector.tensor_scalar(
                            out=jd[p0 : p0 + pn, :nt, :],
                            in0=view[p0 : p0 + pn, :, qi, :],
                            scalar1=0.5, scalar2=0.0, op0=A.mult, op1=A.add,
                            accum_out=dsb[k][p0 : p0 + pn, col : col + 1],
                        )
                        jd2 = smallp.tile(
                            [128, 12, 8], f16, name=f"jd2{k}_{col}_{p0}", tag="jd2"
                        )
                        nc.vector.scalar_tensor_tensor(
                            out=jd2[p0 : p0 + pn, :nt, :],
                            in0=view[p0 : p0 + pn, :, qi, :],
                            scalar=0.5, in1=view[p0 : p0 + pn, :, qi, :],
                            op0=A.mult, op1=A.mult,
                            accum_out=dqb[k][p0 : p0 + pn, col : col + 1],
                        )


            # ================= PASS 1 =================
            gi2tiles = {}
            for ti, (gi, c0, w) in enumerate(TILE_A):
                gi2tiles.setdefault(gi, []).append((ti, c0, w))

            for gi in range(GPC):
                L = _LL[gi]
                o0 = int(_OFF[gi])
                adjs = []
                for pp in range(4):
                    idx = 4 * gi + pp
                    adj = adjp.tile([128, 776], f16, tag="adj", name=f"adj_{idx}")
                    if idx % 3 == 0:
                        nc.scalar.activation(
                            out=adj[:, :L], in_=xw[:, o0 : o0 + L], func=AF.Abs,
                            bias=xp[:, idx : idx + 1], scale=-1.0,
                        )
                    else:
                        d = dtp.tile([128, 776], f16, tag="dt", name=f"d_{idx}")
                        nc.vector.tensor_scalar(
                            out=d[:, :L], in0=xw[:, o0 : o0 + L],
                            scalar1=xp[:, idx : idx + 1], scalar2=None,
                            op0=A.subtract,
                        )
                        nc.vector.scalar_tensor_tensor(
                            out=adj[:, :L], in0=d[:, :L], scalar=-1.0, in1=d[:, :L],
                            op0=A.mult, op1=A.max,
                        )
                    adjs.append(adj)
                for ti, c0, w in gi2tiles[gi]:
                    lc = c0 - o0
                    ps = psA.tile([128, 512], f32, tag="psA", name=f"h1p_{ti}")
                    for pp in range(4):
                        nc.tensor.matmul(
                            ps[32 * pp : 32 * pp + 32, :w],
                            l1[:, :],
                            adjs[pp][:, lc : lc + w],
                            start=True, stop=True,
                            tile_position=(0, 32 * pp),
                        )
                    copy_and_stats(1, ti, ps[:, :w], w, h1[:, c0 : c0 + w], False)
            diag_stats_batched(1, h1, 0)

            # ---- barrier ----
            def barrier(k, pat, gcol, becol):
                nbn, wbn = n_bn[k], w_bn[k]
                ba = smallp.tile([128, 2], f32, name=f"ba{k}")
                nc.vector.bn_aggr(ba[:, :], stbn[k][:, : 6 * nbn])
                m2 = smallp.tile([128, 1], f32, name=f"m2_{k}")
                nc.vector.tensor_tensor(
                    out=m2[:, :], in0=ba[:, 0:1], in1=ba[:, 0:1], op=A.mult,
                )
                q1 = smallp.tile([128, 1], f32, name=f"q1_{k}")
                nc.vector.tensor_tensor(
                    out=q1[:, :], in0=ba[:, 1:2], in1=m2[:, :], op=A.add,
                )
                ds_l = smallp.tile([128, 1], f32, name=f"dsl{k}")
                nc.vector.tensor_reduce(
                    out=ds_l[:, :], in_=dsb[k][:, :],
                    axis=mybir.AxisListType.X, op=A.add,
                )
                dq_l = smallp.tile([128, 1], f32, name=f"dql{k}")
                nc.vector.tensor_reduce(
                    out=dq_l[:, :], in_=dqb[k][:, :],
                    axis=mybir.AxisListType.X, op=A.add,
                )
                # ds_l/dq_l hold DS/2, DQ/2 (0.5-scaled accums). col0 = DS/2 - S ; col1 = Q - DQ/2
                sq = smallp.tile([128, 2], f32, name=f"sq{k}")
                nc.vector.scalar_tensor_tensor(
                    out=sq[:, 0:1], in0=ba[:, 0:1], scalar=float(-wbn), in1=ds_l[:, :],
                    op0=A.mult, op1=A.add,
                )
                nc.vector.scalar_tensor_tensor(
                    out=sq[:, 1:2], in0=q1[:, :], scalar=float(wbn), in1=dq_l[:, :],
                    op0=A.mult, op1=A.subtract,
                )
                pf = psS.tile([128, 2], f32, tag="psS", name=f"pf{k}")
                nc.tensor.matmul(pf[:, :], pat[:, :], sq[:, :], start=True, stop=True)
                gl = smallp.tile([128, 2], f32, name=f"gl{k}")
                nc.vector.tensor_copy(gl[:, :], pf[:, :])
                agi = dram.tile([128, 2], f32, name=f"agi{k}")
                ago = dram.tile([128 * NC_, 2], f32, addr_space="Shared", name=f"ago{k}")
                nc.sync.dma_start(agi[:, :], gl[:, :])
                nc.gpsimd.collective_compute(
                    "AllGather", A.bypass,
                    replica_groups=[list(range(NC_))],
                    ins=[agi.opt()], outs=[ago.opt()],
                )
                agv = smallp.tile([128, 2, NC_], f32, name=f"agv{k}")
                nc.sync.dma_start(
                    agv[:, :, :], ago.rearrange("(b p) c -> p c b", b=NC_),
                )
                gt = smallp.tile([128, 2], f32, name=f"gt{k}")
                nc.vector.tensor_reduce(
                    out=gt[:, :], in_=agv[:, :, :],
                    axis=mybir.AxisListType.X, op=A.add,
                )
                # gt0 = (DS/2 - S)_global ; gt1 = (Q - DQ/2)_global
                negmean = smallp.tile([128, 1], f32, name=f"nm{k}")
                nc.vector.tensor_scalar(
                    out=negmean[:, :], in0=gt[:, 0:1], scalar1=2.0 / NTOT,
                    scalar2=None, op0=A.mult,
                )
                msq = smallp.tile([128, 1], f32, name=f"ms{k}")
                nc.vector.tensor_tensor(
                    out=msq[:, :], in0=negmean[:, :], in1=negmean[:, :], op=A.mult,
                )
                ex2e = smallp.tile([128, 1], f32, name=f"ex{k}")
                nc.vector.tensor_scalar(
                    out=ex2e[:, :], in0=gt[:, 1:2], scalar1=2.0 / NTOT,
                    scalar2=EPS, op0=A.mult, op1=A.add,
                )
                vpe = smallp.tile([128, 1], f32, name=f"vp{k}")
                nc.vector.scalar_tensor_tensor(
                    out=vpe[:, :], in0=msq[:, :], scalar=-1.0, in1=ex2e[:, :],
                    op0=A.mult, op1=A.add,
                )
                rinv = smallp.tile([128, 1], f32, name=f"ri{k}")
                nc.vector.reciprocal(rinv[:, :], vpe[:, :])
                rstd = smallp.tile([128, 1], f32, name=f"rs{k}")
                nc.scalar.activation(out=rstd[:, :], in_=rinv[:, :], func=AF.Sqrt)
                sk = smallp.tile([128, 1], f32, name=f"s{k}")
                nc.vector.tensor_tensor(
                    out=sk[:, :], in0=rstd[:, :], in1=gb[:, gcol : gcol + 1], op=A.mult,
                )
                tk = smallp.tile([128, 1], f32, name=f"t{k}")
                nc.vector.scalar_tensor_tensor(
                    out=tk[:, :], in0=sk[:, :], scalar=negmean[:, :],
                    in1=gb[:, becol : becol + 1], op0=A.mult, op1=A.add,
                )
                return sk, tk

            s1, t1 = barrier(1, p16, 0, 1)

            # ================= PASS 2: apply1, mm2, h2 =================
            h2 = big.tile([128, WTA], f16, tag="hbuf")
            for si, (c0, w) in enumerate(SLAB_A):
                at = atp.tile([128, 1536], f16, tag="at", name=f"a1_{si}")
                nc.scalar.activation(
                    out=at[:, :w], in_=h1[:, c0 : c0 + w],
                    func=AF.Lrelu, scale=s1[:, :], bias=t1[:, :], alpha=SLOPE,
                )
                for z in range(0, w, 512):
                    wz = min(512, w - z)
                    ti = (c0 + z) // 512
                    ps = psA.tile([128, 512], f32, tag="psA", name=f"h2p_{ti}")
                    nc.tensor.matmul(
                        ps[:, :wz], l2[:, :], at[:, z : z + wz],
                        start=True, stop=True,
                    )
                    copy_and_stats(
                        2, ti, ps[:, :wz], wz, h2[:, c0 + z : c0 + z + wz], False
                    )
            diag_stats_batched(2, h2, 0)

            s2, t2 = barrier(2, p16, 2, 3)

            # ================= PASS 3: apply2, mm3, h3 =================
            a2 = big.tile([128, WTA], f16, tag="hbuf")
            for si, (c0, w) in enumerate(SLAB_A):
                if si % 2 == 0:
                    nc.scalar.activation(
                        out=a2[:, c0 : c0 + w], in_=h2[:, c0 : c0 + w],
                        func=AF.Lrelu, scale=s2[:, :], bias=t2[:, :], alpha=SLOPE,
                    )
                else:
                    u = dtp.tile([128, 1536], f16, tag="dt2", name=f"u2_{si}")
                    nc.vector.tensor_scalar(
                        out=u[:, :w], in0=h2[:, c0 : c0 + w], scalar1=s2[:, :],
                        scalar2=t2[:, :], op0=A.mult, op1=A.add,
                    )
                    nc.vector.scalar_tensor_tensor(
                        out=a2[:, c0 : c0 + w], in0=u[:, :w], scalar=SLOPE,
                        in1=u[:, :w], op0=A.mult, op1=A.max,
                    )
            h3 = big.tile([128, WTB], f16, tag="hbuf")
            for ti, (c0, w) in enumerate(TILE_B):
                ps = psB.tile([128, 384], f32, tag="psB", name=f"h3p_{ti}")
                for u in range(2):
                    nc.tensor.matmul(
                        ps[64 * u : 64 * u + 64, :w],
                        l3[:, :],
                        a2[:, WTB * u + c0 : WTB * u + c0 + w],
                        start=True, stop=True,
                        tile_position=(0, 64 * u),
                    )
                copy_and_stats(3, ti, ps[:, :w], w, h3[:, c0 : c0 + w], False)
            diag_stats_batched(3, h3, 1)

            s3, t3v = barrier(3, p8, 4, 5)

            # ================= PASS 4: apply3, mm4, h4 =================
            h4 = big.tile([128, WTB], f16, tag="hbuf")
            for si, (c0, w) in enumerate(SLAB_B):
                at = atp.tile([128, 1536], f16, tag="at", name=f"a3_{si}")
                nc.scalar.activation(
                    out=at[:, :w], in_=h3[:, c0 : c0 + w],
                    func=AF.Lrelu, scale=s3[:, :], bias=t3v[:, :], alpha=SLOPE,
                )
                for z in range(0, w, 384):
                    wz = min(384, w - z)
                    ti = (c0 + z) // 384
                    ps = psB.tile([128, 384], f32, tag="psB", name=f"h4p_{ti}")
                    nc.tensor.matmul(
                        ps[:, :wz], l4[:, :], at[:, z : z + wz],
                        start=True, stop=True,
                    )
                    copy_and_stats(
                        4, ti, ps[:, :wz], wz, h4[:, c0 + z : c0 + z + wz], False
                    )
            diag_stats_batched(4, h4, 1)

            s4, t4v = barrier(4, p8, 6, 7)

            # ================= PASS 5: apply4, mm5, out =================
            outb = outp.tile([128, WOUT], f32)
            for pi in range(NP5):
                ps5 = psB.tile([128, 384], f32, tag="psB", name=f"h5p_{pi}")
                for k in range(4):
                    ti = 4 * pi + k
                    if ti >= NTB:
                        nc.vector.memset(ps5[32 * k : 32 * k + 16, :], 0.0)
                        continue
                    c0, w = TILE_B[ti]
                    at = atp.tile([128, 1536], f16, tag="at", name=f"a4_{ti}")
                    if ti % 3 == 2:
                        u = dtp.tile([128, 1536], f16, tag="dt2", name=f"u4_{ti}")
                        nc.vector.tensor_scalar(
                            out=u[:, :w], in0=h4[:, c0 : c0 + w], scalar1=s4[:, :],
                            scalar2=t4v[:, :], op0=A.mult, op1=A.add,
                        )
                        nc.vector.scalar_tensor_tensor(
                            out=at[:, :w], in0=u[:, :w], scalar=SLOPE,
                            in1=u[:, :w], op0=A.mult, op1=A.max,
                        )
                    else:
                        nc.scalar.activation(
                            out=at[:, :w], in_=h4[:, c0 : c0 + w],
                            func=AF.Lrelu, scale=s4[:, :], bias=t4v[:, :], alpha=SLOPE,
                        )
                    nc.tensor.matmul(
                        ps5[32 * k : 32 * k + 16, :w], l5[:, :], at[:, :w],
                        start=True, stop=True,
                        tile_position=(0, 32 * k),
                    )
                    if w < 384:
                        nc.vector.memset(ps5[32 * k : 32 * k + 16, w:384], 0.0)
                nc.scalar.activation(
                    out=outb[:, 384 * pi : 384 * pi + 384], in_=ps5[:, :],
                    func=AF.Identity, bias=b5b[:, :], scale=1.0,
                )
            nc.sync.dma_start(out_e[:, :], outb[:, :])

    nc.compile()
    return nc


def _host_inputs(x, W1, W2, W3, W4, W5, g1, be1, g2, be2, g3, be3, g4, be4, b5):
    xT = x.T.astype(np.float32)  # [64, 1536]

    lhsT1 = np.zeros((128, 32), np.float32)
    for d in range(2):
        lhsT1[64 * d : 64 * d + 64, 16 * d : 16 * d + 16] = W1.T
    lhsT2 = np.zeros((128, 128), np.float32)
    for r in range(8):
        lhsT2[16 * r : 16 * r + 16, 16 * r : 16 * r + 16] = W2.T
    lhsT3 = np.zeros((128, 64), np.float32)
    for r in range(8):
        lhsT3[16 * r : 16 * r + 16, 8 * r : 8 * r + 8] = W3.T
    lhsT4 = np.zeros((128, 128), np.float32)
    for b in range(16):
        lhsT4[8 * b : 8 * b + 8, 8 * b : 8 * b + 8] = W4.T
    lhsT5 = np.zeros((128, 16), np.float32)
    for b in range(16):
        lhsT5[8 * b : 8 * b + 8, b] = W5[0, :]

    q = np.arange(128)
    pat16 = (q[:, None] % 16 == q[None, :] % 16).astype(np.float32)
    pat8 = (q[:, None] % 8 == q[None, :] % 8).astype(np.float32)
    gb = np.stack(
        [
            g1[q % 16], be1[q % 16], g2[q % 16], be2[q % 16],
            g3[q % 8], be3[q % 8], g4[q % 8], be4[q % 8],
        ],
        axis=1,
    ).astype(np.float32)
    b5b = np.full((128, 1), float(b5[0]), np.float32)

    common = {
        "lhsT1": lhsT1.astype(np.float16),
        "lhsT1n": (-lhsT1).astype(np.float16),
        "lhsT2": lhsT2.astype(np.float16),
        "lhsT3": lhsT3.astype(np.float16),
        "lhsT4": lhsT4.astype(np.float16),
        "lhsT5": lhsT5.astype(np.float16),
        "pat16": pat16,
        "pat8": pat8,
        "gb": gb,
        "b5b": b5b,
    }

    in_maps = []
    for core in range(NC_):
        gl = _glist(core)
        xw = np.empty((64, WTA), np.float32)
        xp = np.zeros((128, 96), np.float32)
        for gi, g in enumerate(gl):
            o0, o1 = int(_OFF[gi]), int(_OFF[gi + 1])
            cols = (8 * g + np.arange(o1 - o0)) % N
            xw[:, o0:o1] = xT[:, cols]
            for pp in range(4):
                for d in range(2):
                    xp[64 * d : 64 * d + 64, 4 * gi + pp] = x[8 * g + 2 * pp + d, :]
        m = dict(common)
        m["xw"] = np.concatenate([xw, xw], axis=0).astype(np.float16)
        m["xp"] = xp
        in_maps.append(m)
    return in_maps


def _decode_maps():
    """Static scatter maps: (core, partition, outcol) -> (row, col) of out[N,N]."""
    if "maps" in _CACHE:
        return _CACHE["maps"]
    rows = np.zeros((NC_, 128, WOUT), np.int32)
    cols = np.zeros((NC_, 128, WOUT), np.int32)
    valid = np.zeros((NC_, 128, WOUT), bool)
    for core in range(NC_):
        gl = _glist(core)
        for ti, (cb, w) in enumerate(TILE_B):
            pi, k = ti // 4, ti % 4
            for u in range(2):
                cA0 = WTB * u + cb
                for gi in range(GPC):
                    lo = max(int(_OFF[gi]), cA0)
                    hi = min(int(_OFF[gi + 1]), cA0 + w)
                    if lo >= hi:
                        continue
                    g = gl[gi]
                    jj = np.arange(lo, hi)
                    j = (8 * g + (jj - int(_OFF[gi]))) % N
                    oc = 384 * pi + (jj - cA0)
                    for r in range(8):
                        p = 32 * k + 8 * u + r
                        rows[core, p, oc] = 8 * g + r
                        cols[core, p, oc] = j
                        valid[core, p, oc] = True
    _CACHE["maps"] = (rows, cols, valid)
    return _CACHE["maps"]


def kernel(**inputs):
    global LAST_EXEC_NS
    import os

    x = np.asarray(inputs["x"], np.float32)
    args = [
        np.asarray(inputs[k], np.float32)
        for k in ("W1", "W2", "W3", "W4", "W5", "g1", "be1", "g2", "be2",
                  "g3", "be3", "g4", "be4", "b5")
    ]
    in_maps = _host_inputs(x, *args)

    if "nc" not in _CACHE:
        _CACHE["nc"] = _build()
    nc = _CACHE["nc"]

    trace = os.environ.get("KERNEL_TRACE", "0") == "1"
    res = run_bass_kernel_spmd(nc, in_maps, core_ids=list(range(NC_)), trace=trace)
    LAST_EXEC_NS = res.exec_time_ns

    rows, cols, valid = _decode_maps()
    out = np.zeros((N, N), np.float32)
    for core in range(NC_):
        raw = np.asarray(res.results[core]["out"])
        v = valid[core]
        out[rows[core][v], cols[core][v]] = raw[v]
    # mirror the uncovered orientations (covered set: every unordered pair once)
    if "mirror" not in _CACHE:
        cov = np.zeros((N, N), bool)
        for core in range(NC_):
            v = valid[core]
            cov[rows[core][v], cols[core][v]] = True
        _CACHE["mirror"] = ~cov
    m = _CACHE["mirror"]
    out[m] = out.T[m]
    return out
